# revision 14
# baseline (speedup 1.0000x reference)
"""Multi-head self-attention (1x1-conv QKV -> softmax attention -> 1x1-conv)
on Trainium2, 8 NeuronCores, sharded by (batch, head-group).

Problem (hardcoded): x[4,256,48,48], Wqkv[768,256], bqkv[768], W0[256,256],
b0[256]; heads=8, dim_head=32, n=2304 pixels.

Sharding: core = b*2 + mg, mg in {0,1} a group of 4 heads. Each core runs
QKV projection (its 4 heads), attention over all 2304 queries x 2304 keys,
and a PARTIAL output projection y_mg = W0[:, mg-heads] @ out_mg (+ b0 on
mg=0 only). The host sums the two partials per image — a pure elementwise
add during output assembly. No cross-core communication on device.

Per-core dataflow — all large matmuls in BF16 (1 cyc/row on the PE vs 4
for fp32; tolerance is 2e-2 so bf16 is plenty):
  - x_aug [257, 2304] bf16: image + ones row, DMA'd in 512-col chunks so
    projections start early.
  - k,q [(4 heads x 32)=128, 2304] bf16 (Wq, bq pre-scaled by d^-0.5 on
    host), vT [j, 4*(32+1)=132] bf16: per head 32 v-dims + ones col
    (bias + softmax denominator via the x ones-row / vt ones-col tricks).
  - scores^T S_T[j, i] per head pair: K=32 bf16 matmuls row-packed via
    tile_position; each matmul output owns a full PSUM bank. Queries in
    chunks (512,512,512,512,256) — wide chunks keep PE array utilization
    high (narrow ones let the HAM clock-gate re-throttle to 1.2 GHz).
  - P = exp(S_T), split across TWO engines per key-tile j:
      * ACT: table exp (exact), bf16 out
      * DVE: Schraudolph bit-trick exp targeting bf16 bits: one
        scalar_tensor_tensor (st*A16 + B16) -> int16 tile, bitcast bf16
        (max elementwise err ~3.4%; mostly cancels post-softmax).
  - out^T+den: the head PAIR's PV matmuls share ONE PSUM bank: a K=1
    "opener" matmul (start=True) writes a pattern row — 0.0 on the output
    rows 0:33/64:97, 1.0 on the junk rows — so both heads can accumulate
    with start=False (pending-zero bytes make their first write an
    overwrite) and the junk rows stay reciprocal-safe. All MMs of a group
    are chained with chain_iter_dep so the scheduler cannot move an
    accumulate past the closing stop.
  - HAM fillers: the PE clock-gate re-throttles unless the array stays
    busy; full-array K=128 filler matmuls over resident tiles run in the
    queue slots where the PV matmul would otherwise sit waiting, and each
    chunk's normalize is deferred into the next chunk's j-loop so the PE
    never drains at a chunk boundary.
  - normalize: dens sit at psum partitions 32/96. Two 32-row block copies
    to a base-0 tile (custom DVE ops need base 0), one
    reciprocal_approx_fast over 64 partitions, recips staged to bf16,
    then a K=64 bf16 mask matmul broadcasts recipA to partitions 0:32 and
    recipB to 64:96 of an rr PSUM bank; ACT stages rr to SBUF (DVE reads
    at most one PSUM operand) and one [128,w] DVE mul writes normalized
    bf16 outc.
  - y_mg = W0_mg @ outc (+ b0 via ones-row matmul on mg=0), fp32 out,
    DMA per chunk.
"""

import os as _os

import numpy as np
import ml_dtypes

import concourse.bass as bass
import concourse.mybir as mybir
import concourse.tile as tile
from concourse import bacc
from concourse import bass_utils

F32 = mybir.dt.float32
BF = mybir.dt.bfloat16
I16 = mybir.dt.int16
AF = mybir.ActivationFunctionType
ALU = mybir.AluOpType
NPBF = ml_dtypes.bfloat16

B, C, HH, WW = 4, 256, 48, 48
HEADS, D = 8, 32
N = HH * WW            # 2304 pixels = queries = keys per core
NCORES = 8
JT = N // 128          # 18 key tiles
MG = 4                 # heads per core
NV = MG * 64           # 256: vT cols/head: 32 v + ones + 31 zeros (PE-array padding)
ICW = 512              # query chunk tile width

# query chunks: wide and uniform; only one narrow tail (narrow chunks drop
# PE array utilization enough for the HAM clock-gate to re-throttle)
QCHUNKS = [(0, 480), (480, 480), (960, 480), (1440, 480), (1920, 384)]

# Schraudolph exp in bf16 bit space: exp(s) ~ bitcast_bf16(int16(A16*s + B16))
A16_SCHR = float((1 << 7) / np.log(2.0))         # 184.665
B16_SCHR = float((127 << 7) - 5.375)             # C16=5.375: ~3.4% max elem err

N_SPIN0 = int(_os.environ.get("KSPIN0", "64"))   # warmup matmuls before projections
N_FILLN = int(_os.environ.get("KFILL", "2"))     # filler matmuls before each PV pair
N_FILL = int(_os.environ.get("KFILLW", "512"))   # filler matmul free-dim cols

N_DVE_J = int(_os.environ.get("KDVE", "8"))      # of JT=18 key tiles on DVE
DVE_JS = frozenset(((2 * i + 1) * JT) // (2 * N_DVE_J) for i in range(N_DVE_J))


def _chunks(total, step):
    out = []
    o = 0
    while o < total:
        w = min(step, total - o)
        out.append((o, w))
        o += w
    return out


def _body(tc, x_d, wq_d, bq_d, wk_d, bk_d, wv_d, w0_d, w0b_d, msk_d, y_d):
    from contextlib import ExitStack

    nc = tc.nc
    with ExitStack() as ctx:
        const = ctx.enter_context(tc.tile_pool(name="const", bufs=1))
        data = ctx.enter_context(tc.tile_pool(name="data", bufs=1))

        # ---------------- load inputs ----------------
        x_sb = [const.tile([128, N], BF, name=f"xa{t}", tag=f"xa{t}") for t in range(2)]
        x1_sb = const.tile([1, N], BF, name="xones", tag="xones")
        for (o, w) in _chunks(N, 512):
            nc.sync.dma_start(x_sb[0][:, o:o + w], x_d[0:128, o:o + w])
            nc.sync.dma_start(x_sb[1][:, o:o + w], x_d[128:256, o:o + w])
        nc.gpsimd.dma_start(x1_sb[:], x_d[256:257, :])

        def load2(name, dram, cols, dt=BF):
            ts_ = [const.tile([128, cols], dt, name=f"{name}{t}", tag=f"{name}{t}") for t in range(2)]
            nc.sync.dma_start(ts_[0][:], dram[0:128, :])
            nc.sync.dma_start(ts_[1][:], dram[128:256, :])
            return ts_

        # wq/wk: [256 chan-in, 256 out] — per head 32 real cols + 32 zero
        # cols, so k/q carry zero-interleaved rows and the K=64 score
        # matmuls light the whole PE array (zeros add nothing)
        wq_sb = load2("wq", wq_d, C)
        wk_sb = load2("wk", wk_d, C)
        wv_sb = load2("wv", wv_d, NV)
        wv1_sb = const.tile([1, NV], BF, name="wvbias", tag="wvbias")
        nc.gpsimd.dma_start(wv1_sb[:], wv_d[256:257, :])
        w0_sb = const.tile([128, C], BF, name="w0", tag="w0")
        nc.sync.dma_start(w0_sb[:], w0_d[0:128, :])
        w01_sb = const.tile([1, C], BF, name="w0bias", tag="w0bias")
        nc.gpsimd.dma_start(w01_sb[:], w0b_d[0:1, :])
        bq_sb = load2("bq", bq_d, 1, dt=F32)
        bk_sb = load2("bk", bk_d, 1, dt=F32)
        mask64 = const.tile([64, 128], BF, name="mask64", tag="mask64")
        nc.gpsimd.dma_start(mask64[:], msk_d[:, :])

        ones_row = const.tile([1, N], BF, name="ones_row", tag="ones_row")
        nc.vector.memset(ones_row[:], 1.0)
        # Schraudolph additive constant, matching the exp input AP shape
        bexp = const.tile([128, 2 * ICW], F32, name="bexp", tag="bexp")
        nc.vector.memset(bexp[:], B16_SCHR)
        # bank-opener row: 0 over the PV output rows (0:33, 64:97), 1.0 over
        # the junk rows so the den blocks stay reciprocal-safe
        patt = const.tile([1, 128], BF, name="patt", tag="patt")
        nc.vector.memset(patt[:], 1.0)
        nc.vector.memset(patt[0:1, 0:33], 0.0)
        nc.vector.memset(patt[0:1, 64:97], 0.0)

        # persistent activations: pair tile g holds [kA, 0, kB, 0] rows
        k_sb = [data.tile([128, N], BF, name=f"k{g}", tag=f"k{g}") for g in range(2)]
        q_sb = [data.tile([128, N], BF, name=f"q{g}", tag=f"q{g}") for g in range(2)]
        vt_sb = [data.tile([128, NV], BF, name=f"vt{j}", tag=f"vt{j}") for j in range(JT)]
        # output tiles in pv layout: tile pr holds head 2*pr at partitions
        # 0-31 and head 2*pr+1 at partitions 64-95
        outc_sb = [data.tile([128, N], BF, name=f"oc{t}", tag=f"oc{t}") for t in range(2)]
        y_sb = [data.tile([128, N], F32, name=f"y{g}", tag=f"y{g}") for g in range(2)]

        # ---------------- projections (bf16) ----------------
        with tc.tile_pool(name="prj", bufs=2, space="PSUM") as prj:
            for g in range(2):
                gsl = slice(g * 128, (g + 1) * 128)
                for (o, w) in _chunks(N, 512):
                    kps = prj.tile([128, 512], F32, name="kps", tag="kps")
                    nc.tensor.matmul(kps[:, :w], wk_sb[0][:, gsl], x_sb[0][:, o:o + w], start=True, stop=False)
                    nc.tensor.matmul(kps[:, :w], wk_sb[1][:, gsl], x_sb[1][:, o:o + w], start=False, stop=True)
                    nc.scalar.activation(k_sb[g][:, o:o + w], kps[:, :w], AF.Identity, bias=bk_sb[g][:, 0:1])
                    qps = prj.tile([128, 512], F32, name="qps", tag="qps")
                    nc.tensor.matmul(qps[:, :w], wq_sb[0][:, gsl], x_sb[0][:, o:o + w], start=True, stop=False)
                    nc.tensor.matmul(qps[:, :w], wq_sb[1][:, gsl], x_sb[1][:, o:o + w], start=False, stop=True)
                    nc.scalar.activation(q_sb[g][:, o:o + w], qps[:, :w], AF.Identity, bias=bq_sb[g][:, 0:1])
            for j in range(JT):
                jsl = slice(j * 128, (j + 1) * 128)
                vps = prj.tile([128, NV], F32, name="vps", tag="vps")
                nc.tensor.matmul(vps[:], x_sb[0][:, jsl], wv_sb[0][:], start=True, stop=False)
                nc.tensor.matmul(vps[:], x_sb[1][:, jsl], wv_sb[1][:], start=False, stop=False)
                nc.tensor.matmul(vps[:], x1_sb[:, jsl], wv1_sb[:], start=False, stop=True)
                nc.scalar.copy(vt_sb[j][:], vps[:])

        # ---------------- attention main loop ----------------
        # PSUM budget: stp 2x2 banks + pvp 2x1 (bank-shared head pair)
        # + rrp 1 + wrm 1 = 8.
        with tc.tile_pool(name="stp", bufs=2, space="PSUM") as stp, \
             tc.tile_pool(name="pvp", bufs=2, space="PSUM") as pvp, \
             tc.tile_pool(name="rrp", bufs=1, space="PSUM") as rrp, \
             tc.tile_pool(name="wrm", bufs=1, space="PSUM") as wrm, \
             tc.tile_pool(name="ptp", bufs=4) as ptp, \
             tc.tile_pool(name="epi", bufs=2) as epi:
            wt = wrm.tile([128, 512], F32, name="wt", tag="wt")

            # HAM filler: full-array K=128/M=128 matmul over resident tiles
            # (output is garbage, never read). Alternate output regions so
            # consecutive fillers carry no write-after-write dependency.
            sp_state = [0]

            def spin(n, cols=N_FILL):
                for _ in range(n):
                    o = 256 * (sp_state[0] & 1)
                    sp_state[0] += 1
                    c = min(cols, 256)
                    nc.tensor.matmul(
                        wt[:, o:o + c], wq_sb[0][:, 0:128], x_sb[0][:, 0:c],
                        start=True, stop=True, tile_position=(0, 0),
                    )

            # warm the PE during the x-DMA wait so projections start at
            # full clock
            spin(N_SPIN0, 256)

            pending = []   # deferred normalize: (pv, oc, ic0, w)
            rcs = {}

            def flush_norm():
                while pending:
                    pv, oc, ic0, w = pending.pop(0)
                    rr = rrp.tile([128, ICW], F32, name="rr", tag="rr")
                    nc.tensor.matmul(
                        rr[:, 0:w], mask64[:, :], rcs.pop(id(pv))[0:64, 0:w],
                        start=True, stop=True, tile_position=(0, 0),
                    )
                    # DVE reads at most one PSUM operand; stage rr in SBUF
                    # via ACT
                    rs = epi.tile([128, ICW], F32, name="rs", tag="rs")
                    nc.scalar.copy(rs[:, 0:w], rr[:, 0:w])
                    nc.vector.tensor_mul(oc[:, ic0:ic0 + w], pv[:, 0:w], rs[:, 0:w])

            for pr in range(2):
                for (ic0, w) in QCHUNKS:
                    ckey = f"pvc{pr}_{ic0}"
                    pv = pvp.tile([128, ICW], F32, name="pv", tag="pv")
                    pts = {}

                    def emit_pv(j, w=w, pv=pv, pts=pts, pr=pr, ckey=ckey):
                        pt = pts.pop(j)
                        for hl, base in ((0, 0), (1, 64)):
                            gh = 2 * pr + hl
                            mi = nc.tensor.matmul(
                                pv[base:base + 64, 0:w],
                                vt_sb[j][:, gh * 64:gh * 64 + 64],
                                pt[:].bitcast(BF)[:, hl * ICW:hl * ICW + w],
                                start=False,
                                stop=(j == JT - 1 and hl == 1),
                                tile_position=(0, base),
                            )
                            tc.chain_iter_dep(ckey, mi.ins)

                    for j in range(JT):
                        st = stp.tile([128, 1024], F32, name="st", tag="st")
                        for hl in range(2):
                            nc.tensor.matmul(
                                st[:, hl * 512:hl * 512 + w],
                                k_sb[pr][hl * 64:(hl + 1) * 64, j * 128:(j + 1) * 128],
                                q_sb[pr][hl * 64:(hl + 1) * 64, ic0:ic0 + w],
                                start=True, stop=True,
                                tile_position=(hl * 64, 0),
                            )
                        st_v = st[:].rearrange("p (s q) -> p s q", s=2)[:, :, 0:w]
                        if j in DVE_JS:
                            pt = ptp.tile([128, 2 * ICW], I16, name="pt", tag="pt")
                            nc.vector.scalar_tensor_tensor(
                                pt[:].rearrange("p (s q) -> p s q", s=2)[:, :, 0:w],
                                st_v, A16_SCHR,
                                bexp[:].rearrange("p (s q) -> p s q", s=2)[:, :, 0:w],
                                ALU.mult, ALU.add,
                            )
                        else:
                            pt = ptp.tile([128, 2 * ICW], BF, name="pt", tag="pt")
                            nc.scalar.activation(
                                pt[:].rearrange("p (s q) -> p s q", s=2)[:, :, 0:w],
                                st_v, AF.Exp,
                            )
                        pts[j] = pt
                        if j == 1:
                            # open the shared bank: K=1 matmul writes the
                            # pattern row to all 128 partitions, start=True
                            mi = nc.tensor.matmul(
                                pv[:, 0:w], patt[0:1, 0:128], ones_row[0:1, 0:w],
                                start=True, stop=False, tile_position=(0, 0),
                            )
                            tc.chain_iter_dep(ckey, mi.ins)
                        if j == 3:
                            # previous chunk's deferred normalize: by now 4
                            # j-iterations of scores/PV sit ahead of the rr
                            # matmul in the PE queue, so the DVE recip chain
                            # it depends on has already drained
                            flush_norm()
                        if j >= 1:
                            spin(N_FILLN if w >= 480 else N_FILLN + 1)
                            emit_pv(j - 1)
                    spin(N_FILLN if w >= 480 else N_FILLN + 1)
                    emit_pv(JT - 1)

                    # normalize part 1 (inline): dens at psum partitions
                    # 32/96, junk rows hold 1.0 from the opener. Block
                    # copies to a base-0 tile (custom DVE ops need base 0),
                    # one reciprocal over 64 partitions, recips staged bf16
                    # for the mask matmul. The mask matmul + mul are
                    # deferred into the next chunk (flush_norm).
                    oc = outc_sb[pr]
                    dd = epi.tile([64, ICW], F32, name="dd", tag="dd")
                    rc = epi.tile([64, ICW], F32, name="rc", tag="rc")
                    nc.vector.tensor_copy(dd[0:32, 0:w], pv[32:64, 0:w])
                    nc.vector.tensor_copy(dd[32:64, 0:w], pv[96:128, 0:w])
                    nc.vector.reciprocal_approx_fast(rc[0:64, 0:w], dd[0:64, 0:w])
                    rcb = epi.tile([64, ICW], BF, name="rcb", tag="rcb")
                    nc.vector.tensor_copy(rcb[0:64, 0:w], rc[0:64, 0:w])
                    rcs[id(pv)] = rcb
                    pending.append((pv, oc, ic0, w))
            flush_norm()

        # ---------------- partial output projection ----------------
        # compact pv-layout outc tiles into one dense head-major [128, i]
        # tile via SBUF->SBUF DMA partition remap, then K=128 matmuls
        # against this head-group's 128 rows of W0. Bias comes via the
        # ones-row matmul (host zeroes it for mg=1 so it is added once).
        od_sb = data.tile([128, N], BF, name="od", tag="od")
        for pr in range(2):
            src = outc_sb[pr]
            nc.sync.dma_start(od_sb[pr * 64:pr * 64 + 32, :], src[0:32, :])
            nc.sync.dma_start(od_sb[pr * 64 + 32:pr * 64 + 64, :], src[64:96, :])
        with tc.tile_pool(name="fin", bufs=2, space="PSUM") as fin:
            for mt in range(2):
                msl = slice(mt * 128, (mt + 1) * 128)
                for (o, w) in _chunks(N, 512):
                    fps = fin.tile([128, 512], F32, name="fps", tag="fps")
                    nc.tensor.matmul(fps[:, :w], w0_sb[:, msl], od_sb[:, o:o + w], start=True, stop=False)
                    nc.tensor.matmul(fps[:, :w], w01_sb[:, msl], ones_row[:, o:o + w], start=False, stop=True)
                    nc.scalar.copy(y_sb[mt][:, o:o + w], fps[:, :w])
                    nc.sync.dma_start(y_d[msl, o:o + w], y_sb[mt][:, o:o + w])


def build_program():
    nc = bacc.Bacc(
        "TRN2",
        target_bir_lowering=False,
        debug=False,
        enable_asserts=False,
        num_devices=NCORES,
    )
    x_d = nc.dram_tensor("x", [C + 1, N], BF, kind="ExternalInput").ap()
    wq_d = nc.dram_tensor("wq", [C, C], BF, kind="ExternalInput").ap()
    bq_d = nc.dram_tensor("bq", [C, 1], F32, kind="ExternalInput").ap()
    wk_d = nc.dram_tensor("wk", [C, C], BF, kind="ExternalInput").ap()
    bk_d = nc.dram_tensor("bk", [C, 1], F32, kind="ExternalInput").ap()
    wv_d = nc.dram_tensor("wv", [C + 1, NV], BF, kind="ExternalInput").ap()
    w0_d = nc.dram_tensor("w0", [128, C], BF, kind="ExternalInput").ap()
    w0b_d = nc.dram_tensor("w0b", [1, C], BF, kind="ExternalInput").ap()
    msk_d = nc.dram_tensor("msk", [64, 128], BF, kind="ExternalInput").ap()
    y_d = nc.dram_tensor("y", [C, N], F32, kind="ExternalOutput").ap()

    with tile.TileContext(nc) as tc:
        _body(tc, x_d, wq_d, bq_d, wk_d, bk_d, wv_d, w0_d, w0b_d, msk_d, y_d)
    nc.compile()
    return nc


_CACHE = {}


def _get_program():
    if "nc" not in _CACHE:
        _CACHE["nc"] = build_program()
    return _CACHE["nc"]


def make_in_maps(x, Wqkv, bqkv, W0, b0):
    f = np.float32
    x = np.asarray(x, f)
    Wqkv = np.asarray(Wqkv, f)
    bqkv = np.asarray(bqkv, f)
    W0 = np.asarray(W0, f)
    b0 = np.asarray(b0, f)

    scale = f(D) ** f(-0.5)
    # channel o = d*24 + k*8 + m ; column layout is head-major (m, d) -> m*32+d
    md = (np.arange(HEADS)[:, None] + 24 * np.arange(D)[None, :]).reshape(-1)
    q_rows, k_rows, v_rows = md + 0, md + 8, md + 16

    wq_full = np.ascontiguousarray((Wqkv[q_rows, :] * scale).T)   # [256, 256]
    bq_full = (bqkv[q_rows] * scale).reshape(-1, 1)
    wk_full = np.ascontiguousarray(Wqkv[k_rows, :].T)
    bk_full = bqkv[k_rows].reshape(-1, 1)
    w0_full = np.ascontiguousarray(W0.T)                          # [c-in, 256]

    msk = np.zeros((64, 128), f)
    msk[0, 0:32] = 1.0
    msk[32, 64:96] = 1.0
    msk = msk.astype(NPBF)

    per_mg = []
    for mg in range(2):
        hsl = slice(mg * 128, (mg + 1) * 128)
        # zero-interleaved layouts: per head 32 real rows/cols then 32
        # zeros, so K=64 / M=64 matmuls engage the full PE array
        wq = np.zeros((C, C), f)
        wk = np.zeros((C, C), f)
        bq = np.zeros((C, 1), f)
        bk = np.zeros((C, 1), f)
        wv = np.zeros((C + 1, NV), f)
        for m in range(MG):
            gm = mg * MG + m
            wq[:, m * 64:m * 64 + 32] = wq_full[:, gm * 32:(gm + 1) * 32]
            wk[:, m * 64:m * 64 + 32] = wk_full[:, gm * 32:(gm + 1) * 32]
            bq[m * 64:m * 64 + 32] = bq_full[gm * 32:(gm + 1) * 32]
            bk[m * 64:m * 64 + 32] = bk_full[gm * 32:(gm + 1) * 32]
            vr = v_rows[gm * D:(gm + 1) * D]
            wv[0:C, m * 64:m * 64 + 32] = Wqkv[vr, :].T
            wv[C, m * 64:m * 64 + 32] = bqkv[vr]
            wv[C, m * 64 + 32] = 1.0
        w0b = b0[None, :] if mg == 0 else np.zeros((1, C), f)
        per_mg.append({
            "wq": wq.astype(NPBF),
            "bq": bq,
            "wk": wk.astype(NPBF),
            "bk": bk,
            "wv": wv.astype(NPBF),
            "w0": np.ascontiguousarray(w0_full[hsl, :]).astype(NPBF),
            "w0b": np.ascontiguousarray(w0b).astype(NPBF),
            "msk": msk,
        })

    maps = []
    for b in range(B):
        x_aug = np.concatenate([x[b].reshape(C, N), np.ones((1, N), f)], axis=0)
        x_bf = np.ascontiguousarray(x_aug).astype(NPBF)
        for mg in range(2):
            maps.append({"x": x_bf, **per_mg[mg]})
    return maps


def assemble_output(ys):
    out = np.empty((B, C, N), np.float32)
    for b in range(B):
        np.add(ys[2 * b], ys[2 * b + 1], out=out[b])
    return out.reshape(B, C, HH, WW)


def run(inputs, trace=False):
    nc = _get_program()
    maps = make_in_maps(**inputs)
    res = bass_utils.run_bass_kernel_spmd(
        nc, maps, core_ids=list(range(NCORES)), trace=trace
    )
    ys = [res.results[c]["y"] for c in range(NCORES)]
    return assemble_output(ys), res.exec_time_ns


def kernel(**inputs):
    out, _ = run(inputs, trace=False)
    return out


# revision 16
# speedup vs baseline: 1.0141x; 1.0141x over previous
"""Multi-head self-attention (1x1-conv QKV -> softmax attention -> 1x1-conv)
on Trainium2, 8 NeuronCores, sharded by (batch, head-group).

Problem (hardcoded): x[4,256,48,48], Wqkv[768,256], bqkv[768], W0[256,256],
b0[256]; heads=8, dim_head=32, n=2304 pixels.

Sharding: core = b*2 + mg, mg in {0,1} a group of 4 heads. Each core runs
QKV projection (its 4 heads), attention over all 2304 queries x 2304 keys,
and a PARTIAL output projection y_mg = W0[:, mg-heads] @ out_mg (+ b0 on
mg=0 only). The host sums the two partials per image — a pure elementwise
add during output assembly. No cross-core communication on device.

Per-core dataflow — all large matmuls in BF16 (1 cyc/row on the PE vs 4
for fp32; tolerance is 2e-2 so bf16 is plenty):
  - x_aug [257, 2304] bf16: image + ones row, DMA'd in 512-col chunks so
    projections start early.
  - k,q [(4 heads x 32)=128, 2304] bf16 (Wq, bq pre-scaled by d^-0.5 on
    host), vT [j, 4*(32+1)=132] bf16: per head 32 v-dims + ones col
    (bias + softmax denominator via the x ones-row / vt ones-col tricks).
  - scores^T S_T[j, i] per head pair: K=32 bf16 matmuls row-packed via
    tile_position; each matmul output owns a full PSUM bank. Queries in
    chunks (512,512,512,512,256) — wide chunks keep PE array utilization
    high (narrow ones let the HAM clock-gate re-throttle to 1.2 GHz).
  - P = exp(S_T), split across TWO engines per key-tile j:
      * ACT: table exp (exact), bf16 out
      * DVE: Schraudolph bit-trick exp targeting bf16 bits: one
        scalar_tensor_tensor (st*A16 + B16) -> int16 tile, bitcast bf16
        (max elementwise err ~3.4%; mostly cancels post-softmax).
  - out^T+den: the head PAIR's PV matmuls share ONE PSUM bank: a K=1
    "opener" matmul (start=True) writes a pattern row — 0.0 on the output
    rows 0:33/64:97, 1.0 on the junk rows — so both heads can accumulate
    with start=False (pending-zero bytes make their first write an
    overwrite) and the junk rows stay reciprocal-safe. All MMs of a group
    are chained with chain_iter_dep so the scheduler cannot move an
    accumulate past the closing stop.
  - HAM fillers: the PE clock-gate re-throttles unless the array stays
    busy; full-array K=128 filler matmuls over resident tiles run in the
    queue slots where the PV matmul would otherwise sit waiting, and each
    chunk's normalize is deferred into the next chunk's j-loop so the PE
    never drains at a chunk boundary.
  - normalize: dens sit at psum partitions 32/96. Two 32-row block copies
    to a base-0 tile (custom DVE ops need base 0), one
    reciprocal_approx_fast over 64 partitions, recips staged to bf16,
    then a K=64 bf16 mask matmul broadcasts recipA to partitions 0:32 and
    recipB to 64:96 of an rr PSUM bank; ACT stages rr to SBUF (DVE reads
    at most one PSUM operand) and one [128,w] DVE mul writes normalized
    bf16 outc.
  - y_mg = W0_mg @ outc (+ b0 via ones-row matmul on mg=0), fp32 out,
    DMA per chunk.
"""

import os as _os

import numpy as np
import ml_dtypes

import concourse.bass as bass
import concourse.mybir as mybir
import concourse.tile as tile
from concourse import bacc
from concourse import bass_utils

F32 = mybir.dt.float32
BF = mybir.dt.bfloat16
I16 = mybir.dt.int16
AF = mybir.ActivationFunctionType
ALU = mybir.AluOpType
NPBF = ml_dtypes.bfloat16

B, C, HH, WW = 4, 256, 48, 48
HEADS, D = 8, 32
N = HH * WW            # 2304 pixels = queries = keys per core
NCORES = 8
JT = N // 128          # 18 key tiles
MG = 4                 # heads per core
NV = MG * 64           # 256: vT cols/head: 32 v + ones + 31 zeros (PE-array padding)
ICW = 512              # query chunk tile width

# query chunks: wide and uniform; only one narrow tail (narrow chunks drop
# PE array utilization enough for the HAM clock-gate to re-throttle)
QCHUNKS = [(0, 480), (480, 480), (960, 480), (1440, 480), (1920, 384)]

# Schraudolph exp in bf16 bit space: exp(s) ~ bitcast_bf16(int16(A16*s + B16))
A16_SCHR = float((1 << 7) / np.log(2.0))         # 184.665
B16_SCHR = float((127 << 7) - 5.375)             # C16=5.375: ~3.4% max elem err

N_SPIN0 = int(_os.environ.get("KSPIN0", "64"))   # warmup matmuls before projections
N_FILLN = int(_os.environ.get("KFILL", "2"))     # filler matmuls before each PV pair
N_FILL = int(_os.environ.get("KFILLW", "512"))   # filler matmul free-dim cols

N_DVE = int(_os.environ.get("KDVE", "15"))       # of 2*JT=36 exp instrs on DVE
DVE_SLOTS = frozenset(((2 * i + 1) * 2 * JT) // (2 * N_DVE) for i in range(N_DVE))
PV_LAG = 4                                       # pv emission lag (j-iterations)


def _chunks(total, step):
    out = []
    o = 0
    while o < total:
        w = min(step, total - o)
        out.append((o, w))
        o += w
    return out


def _body(tc, x_d, wq_d, bq_d, wk_d, bk_d, wv_d, w0_d, w0b_d, msk_d, y_d):
    from contextlib import ExitStack

    nc = tc.nc
    with ExitStack() as ctx:
        const = ctx.enter_context(tc.tile_pool(name="const", bufs=1))
        data = ctx.enter_context(tc.tile_pool(name="data", bufs=1))

        # ---------------- load inputs ----------------
        x_sb = [const.tile([128, N], BF, name=f"xa{t}", tag=f"xa{t}") for t in range(2)]
        x1_sb = const.tile([1, N], BF, name="xones", tag="xones")
        for (o, w) in _chunks(N, 512):
            nc.sync.dma_start(x_sb[0][:, o:o + w], x_d[0:128, o:o + w])
            nc.sync.dma_start(x_sb[1][:, o:o + w], x_d[128:256, o:o + w])
        nc.gpsimd.dma_start(x1_sb[:], x_d[256:257, :])

        def load2(name, dram, cols, dt=BF):
            ts_ = [const.tile([128, cols], dt, name=f"{name}{t}", tag=f"{name}{t}") for t in range(2)]
            nc.sync.dma_start(ts_[0][:], dram[0:128, :])
            nc.sync.dma_start(ts_[1][:], dram[128:256, :])
            return ts_

        # wq/wk: [256 chan-in, 256 out] — per head 32 real cols + 32 zero
        # cols, so k/q carry zero-interleaved rows and the K=64 score
        # matmuls light the whole PE array (zeros add nothing)
        wq_sb = load2("wq", wq_d, C)
        wk_sb = load2("wk", wk_d, C)
        wv_sb = load2("wv", wv_d, NV)
        wv1_sb = const.tile([1, NV], BF, name="wvbias", tag="wvbias")
        nc.gpsimd.dma_start(wv1_sb[:], wv_d[256:257, :])
        w0_sb = const.tile([128, C], BF, name="w0", tag="w0")
        nc.sync.dma_start(w0_sb[:], w0_d[0:128, :])
        w01_sb = const.tile([1, C], BF, name="w0bias", tag="w0bias")
        nc.gpsimd.dma_start(w01_sb[:], w0b_d[0:1, :])
        bq_sb = load2("bq", bq_d, 1, dt=F32)
        bk_sb = load2("bk", bk_d, 1, dt=F32)
        mask64 = const.tile([64, 128], BF, name="mask64", tag="mask64")
        nc.gpsimd.dma_start(mask64[:], msk_d[:, :])

        ones_row = const.tile([1, N], BF, name="ones_row", tag="ones_row")
        nc.vector.memset(ones_row[:], 1.0)
        # Schraudolph additive constant, matching the exp input AP shape
        bexp = const.tile([128, 2 * ICW], F32, name="bexp", tag="bexp")
        nc.vector.memset(bexp[:], B16_SCHR)
        # bank-opener row: 0 over the PV output rows (0:33, 64:97), 1.0 over
        # the junk rows so the den blocks stay reciprocal-safe
        patt = const.tile([1, 128], BF, name="patt", tag="patt")
        nc.vector.memset(patt[:], 1.0)
        nc.vector.memset(patt[0:1, 0:33], 0.0)
        nc.vector.memset(patt[0:1, 64:97], 0.0)

        # persistent activations: pair tile g holds [kA, 0, kB, 0] rows
        k_sb = [data.tile([128, N], BF, name=f"k{g}", tag=f"k{g}") for g in range(2)]
        q_sb = [data.tile([128, N], BF, name=f"q{g}", tag=f"q{g}") for g in range(2)]
        vt_sb = [data.tile([128, NV], BF, name=f"vt{j}", tag=f"vt{j}") for j in range(JT)]
        # output tiles in pv layout: tile pr holds head 2*pr at partitions
        # 0-31 and head 2*pr+1 at partitions 64-95
        outc_sb = [data.tile([128, N], BF, name=f"oc{t}", tag=f"oc{t}") for t in range(2)]
        y_sb = [data.tile([128, N], F32, name=f"y{g}", tag=f"y{g}") for g in range(2)]

        # ---------------- projections (bf16) ----------------
        with tc.tile_pool(name="prj", bufs=2, space="PSUM") as prj:
            # warm the PE during the x-DMA wait so projections start at
            # full clock: full-array matmuls into a scratch psum tile
            wt = prj.tile([128, 512], F32, name="wt", tag="wt")
            for i in range(N_SPIN0):
                o = 256 * (i & 1)
                nc.tensor.matmul(
                    wt[:, o:o + 256], wq_sb[0][:, 0:128], x_sb[0][:, 0:256],
                    start=True, stop=True, tile_position=(0, 0),
                )
            for g in range(2):
                gsl = slice(g * 128, (g + 1) * 128)
                for (o, w) in _chunks(N, 512):
                    kps = prj.tile([128, 512], F32, name="kps", tag="kps")
                    nc.tensor.matmul(kps[:, :w], wk_sb[0][:, gsl], x_sb[0][:, o:o + w], start=True, stop=False)
                    nc.tensor.matmul(kps[:, :w], wk_sb[1][:, gsl], x_sb[1][:, o:o + w], start=False, stop=True)
                    nc.scalar.activation(k_sb[g][:, o:o + w], kps[:, :w], AF.Identity, bias=bk_sb[g][:, 0:1])
                    qps = prj.tile([128, 512], F32, name="qps", tag="qps")
                    nc.tensor.matmul(qps[:, :w], wq_sb[0][:, gsl], x_sb[0][:, o:o + w], start=True, stop=False)
                    nc.tensor.matmul(qps[:, :w], wq_sb[1][:, gsl], x_sb[1][:, o:o + w], start=False, stop=True)
                    nc.scalar.activation(q_sb[g][:, o:o + w], qps[:, :w], AF.Identity, bias=bq_sb[g][:, 0:1])
            for j in range(JT):
                jsl = slice(j * 128, (j + 1) * 128)
                vps = prj.tile([128, NV], F32, name="vps", tag="vps")
                nc.tensor.matmul(vps[:], x_sb[0][:, jsl], wv_sb[0][:], start=True, stop=False)
                nc.tensor.matmul(vps[:], x_sb[1][:, jsl], wv_sb[1][:], start=False, stop=False)
                nc.tensor.matmul(vps[:], x1_sb[:, jsl], wv1_sb[:], start=False, stop=True)
                nc.scalar.copy(vt_sb[j][:], vps[:])

        # ---------------- attention main loop ----------------
        # Both head pairs interleave in one j-loop: the other pair's
        # matmuls fill the PE queue slots where a lone pair would sit
        # waiting on exp, so the PE stays busy with real work and the HAM
        # clock-gate keeps the array at 2.4 GHz. Both exp engines (ACT
        # table exp / DVE Schraudolph) run every j.
        # PSUM: stp 3x2 banks (scores; rr matmuls borrow ring slots)
        # + pvp 2x1 (bank-shared pair each) = 8.
        with tc.tile_pool(name="stp", bufs=3, space="PSUM") as stp, \
             tc.tile_pool(name="pvp", bufs=1, space="PSUM") as pvp, \
             tc.tile_pool(name="ptp", bufs=12) as ptp, \
             tc.tile_pool(name="epi", bufs=4) as epi:

            pending = []   # deferred normalize: (pv, oc, ic0, w)
            rcs = {}

            def flush_norm():
                while pending:
                    pv, oc, ic0, w = pending.pop(0)
                    rr = stp.tile([128, 1024], F32, name="rr", tag="st")
                    nc.tensor.matmul(
                        rr[:, 0:w], mask64[:, :], rcs.pop(id(pv))[0:64, 0:w],
                        start=True, stop=True, tile_position=(0, 0),
                    )
                    # DVE reads at most one PSUM operand; stage rr in SBUF
                    # via ACT
                    rs = epi.tile([128, ICW], F32, name="rs", tag="rs")
                    nc.scalar.copy(rs[:, 0:w], rr[:, 0:w])
                    nc.vector.tensor_mul(oc[:, ic0:ic0 + w], pv[:, 0:w], rs[:, 0:w])

            for (ic0, w) in QCHUNKS:
                pvs = [pvp.tile([128, ICW], F32, name=f"pv{p}", tag=f"pv{p}") for p in range(2)]
                pts = {}

                def emit_pv(j, w=w, pvs=pvs, pts=pts, ic0=ic0):
                    for p in range(2):
                        pt = pts.pop((j, p))
                        for hl, base in ((0, 0), (1, 64)):
                            gh = 2 * p + hl
                            mi = nc.tensor.matmul(
                                pvs[p][base:base + 64, 0:w],
                                vt_sb[j][:, gh * 64:gh * 64 + 64],
                                pt[:].bitcast(BF)[:, hl * ICW:hl * ICW + w],
                                start=False,
                                stop=(j == JT - 1 and hl == 1),
                                tile_position=(0, base),
                            )
                            tc.chain_iter_dep(f"pvc{p}_{ic0}", mi.ins)

                for j in range(JT):
                    for p in range(2):
                        st = stp.tile([128, 1024], F32, name="st", tag="st")
                        for hl in range(2):
                            nc.tensor.matmul(
                                st[:, hl * 512:hl * 512 + w],
                                k_sb[p][hl * 64:(hl + 1) * 64, j * 128:(j + 1) * 128],
                                q_sb[p][hl * 64:(hl + 1) * 64, ic0:ic0 + w],
                                start=True, stop=True,
                                tile_position=(hl * 64, 0),
                            )
                        st_v = st[:].rearrange("p (s q) -> p s q", s=2)[:, :, 0:w]
                        if (2 * j + p) in DVE_SLOTS:
                            pt = ptp.tile([128, 2 * ICW], I16, name="pt", tag="pt")
                            nc.vector.scalar_tensor_tensor(
                                pt[:].rearrange("p (s q) -> p s q", s=2)[:, :, 0:w],
                                st_v, A16_SCHR,
                                bexp[:].rearrange("p (s q) -> p s q", s=2)[:, :, 0:w],
                                ALU.mult, ALU.add,
                            )
                        else:
                            pt = ptp.tile([128, 2 * ICW], BF, name="pt", tag="pt")
                            nc.scalar.activation(
                                pt[:].rearrange("p (s q) -> p s q", s=2)[:, :, 0:w],
                                st_v, AF.Exp,
                            )
                        pts[(j, p)] = pt
                    if j == 1:
                        # previous chunk's deferred normalize: the j0/j1
                        # score sets ahead of the rr matmuls cover the DVE
                        # recip chain latency
                        flush_norm()
                    if j == 3:
                        # open the shared banks: K=1 matmuls write the
                        # pattern row to all 128 partitions, start=True.
                        # Late enough that the pool-release wait (previous
                        # chunk's normalize muls) is already satisfied.
                        for p in range(2):
                            mi = nc.tensor.matmul(
                                pvs[p][:, 0:w], patt[0:1, 0:128], ones_row[0:1, 0:w],
                                start=True, stop=False, tile_position=(0, 0),
                            )
                            tc.chain_iter_dep(f"pvc{p}_{ic0}", mi.ins)
                    if j >= PV_LAG:
                        emit_pv(j - PV_LAG)
                for j in range(JT - PV_LAG, JT):
                    emit_pv(j)

                # normalize part 1 (inline): dens at psum partitions 32/96,
                # junk rows hold 1.0 from the opener (PV zero-padding
                # accumulates zeros onto them). Block copies to a base-0
                # tile (custom DVE ops need base 0), one
                # reciprocal_approx_fast over 64 partitions, recips staged
                # bf16 (via ACT, for the bf16 mask matmul). The mask matmul
                # + mul are deferred into the next chunk (flush_norm).
                for p in range(2):
                    pv = pvs[p]
                    dd = epi.tile([64, ICW], F32, name="dd", tag="dd")
                    rc = epi.tile([64, ICW], F32, name="rc", tag="rc")
                    nc.vector.tensor_copy(dd[0:32, 0:w], pv[32:64, 0:w])
                    nc.vector.tensor_copy(dd[32:64, 0:w], pv[96:128, 0:w])
                    nc.vector.reciprocal_approx_fast(rc[0:64, 0:w], dd[0:64, 0:w])
                    rcb = epi.tile([64, ICW], BF, name="rcb", tag="rcb")
                    nc.scalar.copy(rcb[0:64, 0:w], rc[0:64, 0:w])
                    rcs[id(pv)] = rcb
                    pending.append((pv, outc_sb[p], ic0, w))
            flush_norm()

        # ---------------- partial output projection ----------------
        # compact pv-layout outc tiles into one dense head-major [128, i]
        # tile via SBUF->SBUF DMA partition remap, then K=128 matmuls
        # against this head-group's 128 rows of W0. Bias comes via the
        # ones-row matmul (host zeroes it for mg=1 so it is added once).
        od_sb = data.tile([128, N], BF, name="od", tag="od")
        for pr in range(2):
            src = outc_sb[pr]
            nc.sync.dma_start(od_sb[pr * 64:pr * 64 + 32, :], src[0:32, :])
            nc.sync.dma_start(od_sb[pr * 64 + 32:pr * 64 + 64, :], src[64:96, :])
        with tc.tile_pool(name="fin", bufs=2, space="PSUM") as fin:
            for mt in range(2):
                msl = slice(mt * 128, (mt + 1) * 128)
                for (o, w) in _chunks(N, 512):
                    fps = fin.tile([128, 512], F32, name="fps", tag="fps")
                    nc.tensor.matmul(fps[:, :w], w0_sb[:, msl], od_sb[:, o:o + w], start=True, stop=False)
                    nc.tensor.matmul(fps[:, :w], w01_sb[:, msl], ones_row[:, o:o + w], start=False, stop=True)
                    nc.scalar.copy(y_sb[mt][:, o:o + w], fps[:, :w])
                    nc.sync.dma_start(y_d[msl, o:o + w], y_sb[mt][:, o:o + w])


def build_program():
    nc = bacc.Bacc(
        "TRN2",
        target_bir_lowering=False,
        debug=False,
        enable_asserts=False,
        num_devices=NCORES,
    )
    x_d = nc.dram_tensor("x", [C + 1, N], BF, kind="ExternalInput").ap()
    wq_d = nc.dram_tensor("wq", [C, C], BF, kind="ExternalInput").ap()
    bq_d = nc.dram_tensor("bq", [C, 1], F32, kind="ExternalInput").ap()
    wk_d = nc.dram_tensor("wk", [C, C], BF, kind="ExternalInput").ap()
    bk_d = nc.dram_tensor("bk", [C, 1], F32, kind="ExternalInput").ap()
    wv_d = nc.dram_tensor("wv", [C + 1, NV], BF, kind="ExternalInput").ap()
    w0_d = nc.dram_tensor("w0", [128, C], BF, kind="ExternalInput").ap()
    w0b_d = nc.dram_tensor("w0b", [1, C], BF, kind="ExternalInput").ap()
    msk_d = nc.dram_tensor("msk", [64, 128], BF, kind="ExternalInput").ap()
    y_d = nc.dram_tensor("y", [C, N], F32, kind="ExternalOutput").ap()

    with tile.TileContext(nc) as tc:
        _body(tc, x_d, wq_d, bq_d, wk_d, bk_d, wv_d, w0_d, w0b_d, msk_d, y_d)
    nc.compile()
    return nc


_CACHE = {}


def _get_program():
    if "nc" not in _CACHE:
        _CACHE["nc"] = build_program()
    return _CACHE["nc"]


def make_in_maps(x, Wqkv, bqkv, W0, b0):
    f = np.float32
    x = np.asarray(x, f)
    Wqkv = np.asarray(Wqkv, f)
    bqkv = np.asarray(bqkv, f)
    W0 = np.asarray(W0, f)
    b0 = np.asarray(b0, f)

    scale = f(D) ** f(-0.5)
    # channel o = d*24 + k*8 + m ; column layout is head-major (m, d) -> m*32+d
    md = (np.arange(HEADS)[:, None] + 24 * np.arange(D)[None, :]).reshape(-1)
    q_rows, k_rows, v_rows = md + 0, md + 8, md + 16

    wq_full = np.ascontiguousarray((Wqkv[q_rows, :] * scale).T)   # [256, 256]
    bq_full = (bqkv[q_rows] * scale).reshape(-1, 1)
    wk_full = np.ascontiguousarray(Wqkv[k_rows, :].T)
    bk_full = bqkv[k_rows].reshape(-1, 1)
    w0_full = np.ascontiguousarray(W0.T)                          # [c-in, 256]

    msk = np.zeros((64, 128), f)
    msk[0, 0:32] = 1.0
    msk[32, 64:96] = 1.0
    msk = msk.astype(NPBF)

    per_mg = []
    for mg in range(2):
        hsl = slice(mg * 128, (mg + 1) * 128)
        # zero-interleaved layouts: per head 32 real rows/cols then 32
        # zeros, so K=64 / M=64 matmuls engage the full PE array
        wq = np.zeros((C, C), f)
        wk = np.zeros((C, C), f)
        bq = np.zeros((C, 1), f)
        bk = np.zeros((C, 1), f)
        wv = np.zeros((C + 1, NV), f)
        for m in range(MG):
            gm = mg * MG + m
            wq[:, m * 64:m * 64 + 32] = wq_full[:, gm * 32:(gm + 1) * 32]
            wk[:, m * 64:m * 64 + 32] = wk_full[:, gm * 32:(gm + 1) * 32]
            bq[m * 64:m * 64 + 32] = bq_full[gm * 32:(gm + 1) * 32]
            bk[m * 64:m * 64 + 32] = bk_full[gm * 32:(gm + 1) * 32]
            vr = v_rows[gm * D:(gm + 1) * D]
            wv[0:C, m * 64:m * 64 + 32] = Wqkv[vr, :].T
            wv[C, m * 64:m * 64 + 32] = bqkv[vr]
            wv[C, m * 64 + 32] = 1.0
        w0b = b0[None, :] if mg == 0 else np.zeros((1, C), f)
        per_mg.append({
            "wq": wq.astype(NPBF),
            "bq": bq,
            "wk": wk.astype(NPBF),
            "bk": bk,
            "wv": wv.astype(NPBF),
            "w0": np.ascontiguousarray(w0_full[hsl, :]).astype(NPBF),
            "w0b": np.ascontiguousarray(w0b).astype(NPBF),
            "msk": msk,
        })

    maps = []
    for b in range(B):
        x_aug = np.concatenate([x[b].reshape(C, N), np.ones((1, N), f)], axis=0)
        x_bf = np.ascontiguousarray(x_aug).astype(NPBF)
        for mg in range(2):
            maps.append({"x": x_bf, **per_mg[mg]})
    return maps


def assemble_output(ys):
    out = np.empty((B, C, N), np.float32)
    for b in range(B):
        np.add(ys[2 * b], ys[2 * b + 1], out=out[b])
    return out.reshape(B, C, HH, WW)


def run(inputs, trace=False):
    nc = _get_program()
    maps = make_in_maps(**inputs)
    res = bass_utils.run_bass_kernel_spmd(
        nc, maps, core_ids=list(range(NCORES)), trace=trace
    )
    ys = [res.results[c]["y"] for c in range(NCORES)]
    return assemble_output(ys), res.exec_time_ns


def kernel(**inputs):
    out, _ = run(inputs, trace=False)
    return out


# revision 17
# speedup vs baseline: 1.2018x; 1.1851x over previous
"""Multi-head self-attention (1x1-conv QKV -> softmax attention -> 1x1-conv)
on Trainium2, 8 NeuronCores, sharded by (batch, head-group).

Problem (hardcoded): x[4,256,48,48], Wqkv[768,256], bqkv[768], W0[256,256],
b0[256]; heads=8, dim_head=32, n=2304 pixels.

Sharding: core = b*2 + mg, mg in {0,1} a group of 4 heads. Each core runs
QKV projection (its 4 heads), attention over all 2304 queries x 2304 keys,
and a PARTIAL output projection y_mg = W0[:, mg-heads] @ out_mg (+ b0 on
mg=0 only). The host sums the two partials per image — a pure elementwise
add during output assembly. No cross-core communication on device.

Per-core dataflow — all large matmuls in BF16 (1 cyc/row on the PE vs 4
for fp32; tolerance is 2e-2 so bf16 is plenty):
  - x_aug [257, 2304] bf16: image + ones row, DMA'd in 512-col chunks so
    projections start early.
  - k,q [(4 heads x 32)=128, 2304] bf16 (Wq, bq pre-scaled by d^-0.5 on
    host), vT [j, 4*(32+1)=132] bf16: per head 32 v-dims + ones col
    (bias + softmax denominator via the x ones-row / vt ones-col tricks).
  - scores^T S_T[j, i] per head pair: K=32 bf16 matmuls row-packed via
    tile_position; each matmul output owns a full PSUM bank. Queries in
    chunks (512,512,512,512,256) — wide chunks keep PE array utilization
    high (narrow ones let the HAM clock-gate re-throttle to 1.2 GHz).
  - P = exp(S_T), split across TWO engines per key-tile j:
      * ACT: table exp (exact), bf16 out
      * DVE: Schraudolph bit-trick exp targeting bf16 bits: one
        scalar_tensor_tensor (st*A16 + B16) -> int16 tile, bitcast bf16
        (max elementwise err ~3.4%; mostly cancels post-softmax).
  - out^T+den: the head PAIR's PV matmuls share ONE PSUM bank: a K=1
    "opener" matmul (start=True) writes a pattern row — 0.0 on the output
    rows 0:33/64:97, 1.0 on the junk rows — so both heads can accumulate
    with start=False (pending-zero bytes make their first write an
    overwrite) and the junk rows stay reciprocal-safe. All MMs of a group
    are chained with chain_iter_dep so the scheduler cannot move an
    accumulate past the closing stop.
  - HAM fillers: the PE clock-gate re-throttles unless the array stays
    busy; full-array K=128 filler matmuls over resident tiles run in the
    queue slots where the PV matmul would otherwise sit waiting, and each
    chunk's normalize is deferred into the next chunk's j-loop so the PE
    never drains at a chunk boundary.
  - normalize: dens sit at psum partitions 32/96. Two 32-row block copies
    to a base-0 tile (custom DVE ops need base 0), one
    reciprocal_approx_fast over 64 partitions, recips staged to bf16,
    then a K=64 bf16 mask matmul broadcasts recipA to partitions 0:32 and
    recipB to 64:96 of an rr PSUM bank; ACT stages rr to SBUF (DVE reads
    at most one PSUM operand) and one [128,w] DVE mul writes normalized
    bf16 outc.
  - y_mg = W0_mg @ outc (+ b0 via ones-row matmul on mg=0), fp32 out,
    DMA per chunk.
"""

import os as _os

import numpy as np
import ml_dtypes

import concourse.bass as bass
import concourse.mybir as mybir
import concourse.tile as tile
from concourse import bacc
from concourse import bass_utils

F32 = mybir.dt.float32
BF = mybir.dt.bfloat16
I16 = mybir.dt.int16
AF = mybir.ActivationFunctionType
ALU = mybir.AluOpType
NPBF = ml_dtypes.bfloat16

B, C, HH, WW = 4, 256, 48, 48
HEADS, D = 8, 32
N = HH * WW            # 2304 pixels = queries = keys per core
NCORES = 8
JT = N // 128          # 18 key tiles
MG = 4                 # heads per core
NV = MG * 64           # 256: vT cols/head: 32 v + ones + 31 zeros (PE-array padding)
ICW = 512              # query chunk tile width

# query chunks: wide and uniform; only one narrow tail (narrow chunks drop
# PE array utilization enough for the HAM clock-gate to re-throttle)
QCHUNKS = [(0, 480), (480, 480), (960, 480), (1440, 480), (1920, 384)]

# Schraudolph exp in bf16 bit space: exp(s) ~ bitcast_bf16(int16(A16*s + B16))
A16_SCHR = float((1 << 7) / np.log(2.0))         # 184.665
B16_SCHR = float((127 << 7) - 5.375)             # C16=5.375: ~3.4% max elem err

N_SPIN0 = int(_os.environ.get("KSPIN0", "16"))   # warmup matmuls before projections
N_FILLN = int(_os.environ.get("KFILL", "2"))     # filler matmuls before each PV pair
N_FILL = int(_os.environ.get("KFILLW", "512"))   # filler matmul free-dim cols

N_DVE = int(_os.environ.get("KDVE", "14"))       # of 2*JT=36 exp instrs on DVE
# keep early slots on ACT: each chunk's DVE queue first drains the previous
# chunk's normalize ops
DVE_SLOTS = frozenset(6 + ((2 * i + 1) * (2 * JT - 6)) // (2 * N_DVE) for i in range(N_DVE))
PV_LAG = 4                                       # pv emission lag (j-iterations)


def _chunks(total, step):
    out = []
    o = 0
    while o < total:
        w = min(step, total - o)
        out.append((o, w))
        o += w
    return out


def _body(tc, x_d, wq_d, bq_d, wk_d, bk_d, wv_d, w0_d, w0b_d, y_d):
    from contextlib import ExitStack

    nc = tc.nc
    with ExitStack() as ctx:
        const = ctx.enter_context(tc.tile_pool(name="const", bufs=1))
        data = ctx.enter_context(tc.tile_pool(name="data", bufs=1))

        # ---------------- load inputs ----------------
        x_sb = [const.tile([128, N], BF, name=f"xa{t}", tag=f"xa{t}") for t in range(2)]
        x1_sb = const.tile([1, N], BF, name="xones", tag="xones")
        for (o, w) in _chunks(N, 512):
            nc.sync.dma_start(x_sb[0][:, o:o + w], x_d[0:128, o:o + w])
            nc.sync.dma_start(x_sb[1][:, o:o + w], x_d[128:256, o:o + w])
        nc.gpsimd.dma_start(x1_sb[:], x_d[256:257, :])

        def load2(name, dram, cols, dt=BF):
            ts_ = [const.tile([128, cols], dt, name=f"{name}{t}", tag=f"{name}{t}") for t in range(2)]
            nc.sync.dma_start(ts_[0][:], dram[0:128, :])
            nc.sync.dma_start(ts_[1][:], dram[128:256, :])
            return ts_

        # wq/wk: [256 chan-in, 256 out] — per head 32 real cols + 32 zero
        # cols, so k/q carry zero-interleaved rows and the K=64 score
        # matmuls light the whole PE array (zeros add nothing)
        wq_sb = load2("wq", wq_d, C)
        wk_sb = load2("wk", wk_d, C)
        wv_sb = load2("wv", wv_d, NV)
        wv1_sb = const.tile([1, NV], BF, name="wvbias", tag="wvbias")
        nc.gpsimd.dma_start(wv1_sb[:], wv_d[256:257, :])
        w0_sb = const.tile([128, C], BF, name="w0", tag="w0")
        nc.sync.dma_start(w0_sb[:], w0_d[0:128, :])
        w01_sb = const.tile([1, C], BF, name="w0bias", tag="w0bias")
        nc.gpsimd.dma_start(w01_sb[:], w0b_d[0:1, :])
        bq_sb = load2("bq", bq_d, 1, dt=F32)
        bk_sb = load2("bk", bk_d, 1, dt=F32)

        ones_row = const.tile([1, N], BF, name="ones_row", tag="ones_row")
        nc.vector.memset(ones_row[:], 1.0)
        # Schraudolph additive constant, matching the exp input AP shape
        bexp = const.tile([128, 2 * ICW], F32, name="bexp", tag="bexp")
        nc.vector.memset(bexp[:], B16_SCHR)
        # bank-opener row: 0 over the PV output rows (0:33, 64:97), 1.0 over
        # the junk rows so the den blocks stay reciprocal-safe
        patt = const.tile([1, 128], BF, name="patt", tag="patt")
        nc.vector.memset(patt[:], 1.0)
        nc.vector.memset(patt[0:1, 0:33], 0.0)
        nc.vector.memset(patt[0:1, 64:97], 0.0)

        # persistent activations: pair tile g holds [kA, 0, kB, 0] rows
        k_sb = [data.tile([128, N], BF, name=f"k{g}", tag=f"k{g}") for g in range(2)]
        q_sb = [data.tile([128, N], BF, name=f"q{g}", tag=f"q{g}") for g in range(2)]
        vt_sb = [data.tile([128, NV], BF, name=f"vt{j}", tag=f"vt{j}") for j in range(JT)]
        # output tiles in pv layout: tile pr holds head 2*pr at partitions
        # 0-31 and head 2*pr+1 at partitions 64-95
        outc_sb = [data.tile([128, N], BF, name=f"oc{t}", tag=f"oc{t}") for t in range(2)]
        y_sb = [data.tile([128, N], F32, name=f"y{g}", tag=f"y{g}") for g in range(2)]

        # ---------------- projections (bf16) ----------------
        with tc.tile_pool(name="prj", bufs=2, space="PSUM") as prj:
            # warm the PE during the x-DMA wait so projections start at
            # full clock: full-array matmuls into a scratch psum tile
            wt = prj.tile([128, 512], F32, name="wt", tag="wt")
            for i in range(N_SPIN0):
                o = 256 * (i & 1)
                nc.tensor.matmul(
                    wt[:, o:o + 256], wq_sb[0][:, 0:128], x_sb[0][:, 0:256],
                    start=True, stop=True, tile_position=(0, 0),
                )
            for g in range(2):
                gsl = slice(g * 128, (g + 1) * 128)
                for (o, w) in _chunks(N, 512):
                    kps = prj.tile([128, 512], F32, name="kps", tag="kps")
                    nc.tensor.matmul(kps[:, :w], wk_sb[0][:, gsl], x_sb[0][:, o:o + w], start=True, stop=False)
                    nc.tensor.matmul(kps[:, :w], wk_sb[1][:, gsl], x_sb[1][:, o:o + w], start=False, stop=True)
                    nc.scalar.activation(k_sb[g][:, o:o + w], kps[:, :w], AF.Identity, bias=bk_sb[g][:, 0:1])
                    qps = prj.tile([128, 512], F32, name="qps", tag="qps")
                    nc.tensor.matmul(qps[:, :w], wq_sb[0][:, gsl], x_sb[0][:, o:o + w], start=True, stop=False)
                    nc.tensor.matmul(qps[:, :w], wq_sb[1][:, gsl], x_sb[1][:, o:o + w], start=False, stop=True)
                    nc.scalar.activation(q_sb[g][:, o:o + w], qps[:, :w], AF.Identity, bias=bq_sb[g][:, 0:1])
            for j in range(JT):
                jsl = slice(j * 128, (j + 1) * 128)
                vps = prj.tile([128, NV], F32, name="vps", tag="vps")
                nc.tensor.matmul(vps[:], x_sb[0][:, jsl], wv_sb[0][:], start=True, stop=False)
                nc.tensor.matmul(vps[:], x_sb[1][:, jsl], wv_sb[1][:], start=False, stop=False)
                nc.tensor.matmul(vps[:], x1_sb[:, jsl], wv1_sb[:], start=False, stop=True)
                if j % 2 == 0:
                    nc.scalar.copy(vt_sb[j][:], vps[:])
                else:
                    nc.vector.tensor_copy(vt_sb[j][:], vps[:])

        # ---------------- attention main loop ----------------
        # Both head pairs interleave in one j-loop: the other pair's
        # matmuls fill the PE queue slots where a lone pair would sit
        # waiting on exp, so the PE stays busy with real work and the HAM
        # clock-gate keeps the array at 2.4 GHz. Both exp engines (ACT
        # table exp / DVE Schraudolph) run every j.
        # PSUM: stp 3x2 banks (scores; rr matmuls borrow ring slots)
        # + pvp 2x1 (bank-shared pair each) = 8.
        with tc.tile_pool(name="stp", bufs=3, space="PSUM") as stp, \
             tc.tile_pool(name="pvp", bufs=1, space="PSUM") as pvp, \
             tc.tile_pool(name="ptp", bufs=12) as ptp, \
             tc.tile_pool(name="epi", bufs=4) as epi:

            pending = []   # deferred normalize muls: list of closures

            def flush_norm():
                while pending:
                    pending.pop(0)()

            for (ic0, w) in QCHUNKS:
                pvs = [pvp.tile([128, ICW], F32, name=f"pv{p}", tag=f"pv{p}") for p in range(2)]
                pts = {}

                def emit_pv(j, w=w, pvs=pvs, pts=pts, ic0=ic0):
                    for p in range(2):
                        pt = pts.pop((j, p))
                        for hl, base in ((0, 0), (1, 64)):
                            gh = 2 * p + hl
                            mi = nc.tensor.matmul(
                                pvs[p][base:base + 64, 0:w],
                                vt_sb[j][:, gh * 64:gh * 64 + 64],
                                pt[:].bitcast(BF)[:, hl * ICW:hl * ICW + w],
                                start=False,
                                stop=(j == JT - 1 and hl == 1),
                                tile_position=(0, base),
                            )
                            tc.chain_iter_dep(f"pvc{p}_{ic0}", mi.ins)

                for j in range(JT):
                    for p in range(2):
                        st = stp.tile([128, 1024], F32, name="st", tag="st")
                        for hl in range(2):
                            nc.tensor.matmul(
                                st[:, hl * 512:hl * 512 + w],
                                k_sb[p][hl * 64:(hl + 1) * 64, j * 128:(j + 1) * 128],
                                q_sb[p][hl * 64:(hl + 1) * 64, ic0:ic0 + w],
                                start=True, stop=True,
                                tile_position=(hl * 64, 0),
                            )
                        st_v = st[:].rearrange("p (s q) -> p s q", s=2)[:, :, 0:w]
                        if (2 * j + p) in DVE_SLOTS:
                            pt = ptp.tile([128, 2 * ICW], I16, name="pt", tag="pt")
                            nc.vector.scalar_tensor_tensor(
                                pt[:].rearrange("p (s q) -> p s q", s=2)[:, :, 0:w],
                                st_v, A16_SCHR,
                                bexp[:].rearrange("p (s q) -> p s q", s=2)[:, :, 0:w],
                                ALU.mult, ALU.add,
                            )
                        else:
                            pt = ptp.tile([128, 2 * ICW], BF, name="pt", tag="pt")
                            nc.scalar.activation(
                                pt[:].rearrange("p (s q) -> p s q", s=2)[:, :, 0:w],
                                st_v, AF.Exp,
                            )
                        pts[(j, p)] = pt
                    if j == 1:
                        # previous chunk's deferred normalize: the j0/j1
                        # score sets ahead of the rr matmuls cover the DVE
                        # recip chain latency
                        flush_norm()
                    if j == 3:
                        # open the shared banks: K=1 matmuls write the
                        # pattern row to all 128 partitions, start=True.
                        # Late enough that the pool-release wait (previous
                        # chunk's normalize muls) is already satisfied.
                        for p in range(2):
                            mi = nc.tensor.matmul(
                                pvs[p][:, 0:w], patt[0:1, 0:128], ones_row[0:1, 0:w],
                                start=True, stop=False, tile_position=(0, 0),
                            )
                            tc.chain_iter_dep(f"pvc{p}_{ic0}", mi.ins)
                    if j >= PV_LAG:
                        emit_pv(j - PV_LAG)
                for j in range(JT - PV_LAG, JT):
                    emit_pv(j)

                # normalize (all DVE/no PSUM pressure): dens at psum
                # partitions 32/96, junk rows hold 1.0 from the opener (PV
                # zero-padding accumulates zeros onto them). Block copies
                # to a base-0 tile (custom DVE ops need base 0), one
                # reciprocal over 64 partitions, then intra-block
                # stream_shuffle broadcasts + two muls per pair. The muls
                # (which free the pv banks) are deferred into the next
                # chunk so the j-loop tail keeps both engines busy.
                for p in range(2):
                    pv = pvs[p]
                    oc = outc_sb[p]
                    dd = epi.tile([64, ICW], F32, name="dd", tag="dd")
                    rc = epi.tile([64, ICW], F32, name="rc", tag="rc")
                    rba = epi.tile([128, ICW], F32, name="rba", tag="rba")
                    nc.vector.tensor_copy(dd[0:32, 0:w], pv[32:64, 0:w])
                    nc.vector.tensor_copy(dd[32:64, 0:w], pv[96:128, 0:w])
                    nc.vector.reciprocal_approx_fast(rc[0:64, 0:w], dd[0:64, 0:w])
                    nc.vector.stream_shuffle(rba[0:32, 0:w], rc[0:32, 0:w], [0] * 32)
                    nc.vector.stream_shuffle(rba[32:64, 0:w], rc[32:64, 0:w], [0] * 32)
                    nc.vector.tensor_copy(rba[64:96, 0:w], rba[32:64, 0:w])

                    def mul_norm(pv=pv, oc=oc, rba=rba, ic0=ic0, w=w):
                        nc.vector.tensor_mul(oc[0:33, ic0:ic0 + w], pv[0:33, 0:w], rba[0:33, 0:w])
                        nc.vector.tensor_mul(oc[64:97, ic0:ic0 + w], pv[64:97, 0:w], rba[64:97, 0:w])
                    pending.append(mul_norm)
            flush_norm()

        # ---------------- partial output projection ----------------
        # compact pv-layout outc tiles into one dense head-major [128, i]
        # tile via SBUF->SBUF DMA partition remap, then K=128 matmuls
        # against this head-group's 128 rows of W0. Bias comes via the
        # ones-row matmul (host zeroes it for mg=1 so it is added once).
        od_sb = data.tile([128, N], BF, name="od", tag="od")
        for pr in range(2):
            src = outc_sb[pr]
            nc.sync.dma_start(od_sb[pr * 64:pr * 64 + 32, :], src[0:32, :])
            nc.sync.dma_start(od_sb[pr * 64 + 32:pr * 64 + 64, :], src[64:96, :])
        with tc.tile_pool(name="fin", bufs=2, space="PSUM") as fin:
            for mt in range(2):
                msl = slice(mt * 128, (mt + 1) * 128)
                for (o, w) in _chunks(N, 512):
                    fps = fin.tile([128, 512], F32, name="fps", tag="fps")
                    nc.tensor.matmul(fps[:, :w], w0_sb[:, msl], od_sb[:, o:o + w], start=True, stop=False)
                    nc.tensor.matmul(fps[:, :w], w01_sb[:, msl], ones_row[:, o:o + w], start=False, stop=True)
                    nc.scalar.copy(y_sb[mt][:, o:o + w], fps[:, :w])
                    nc.sync.dma_start(y_d[msl, o:o + w], y_sb[mt][:, o:o + w])


def build_program():
    nc = bacc.Bacc(
        "TRN2",
        target_bir_lowering=False,
        debug=False,
        enable_asserts=False,
        num_devices=NCORES,
    )
    x_d = nc.dram_tensor("x", [C + 1, N], BF, kind="ExternalInput").ap()
    wq_d = nc.dram_tensor("wq", [C, C], BF, kind="ExternalInput").ap()
    bq_d = nc.dram_tensor("bq", [C, 1], F32, kind="ExternalInput").ap()
    wk_d = nc.dram_tensor("wk", [C, C], BF, kind="ExternalInput").ap()
    bk_d = nc.dram_tensor("bk", [C, 1], F32, kind="ExternalInput").ap()
    wv_d = nc.dram_tensor("wv", [C + 1, NV], BF, kind="ExternalInput").ap()
    w0_d = nc.dram_tensor("w0", [128, C], BF, kind="ExternalInput").ap()
    w0b_d = nc.dram_tensor("w0b", [1, C], BF, kind="ExternalInput").ap()
    y_d = nc.dram_tensor("y", [C, N], F32, kind="ExternalOutput").ap()

    with tile.TileContext(nc) as tc:
        _body(tc, x_d, wq_d, bq_d, wk_d, bk_d, wv_d, w0_d, w0b_d, y_d)
    nc.compile()
    return nc


_CACHE = {}


def _get_program():
    if "nc" not in _CACHE:
        _CACHE["nc"] = build_program()
    return _CACHE["nc"]


def make_in_maps(x, Wqkv, bqkv, W0, b0):
    f = np.float32
    x = np.asarray(x, f)
    Wqkv = np.asarray(Wqkv, f)
    bqkv = np.asarray(bqkv, f)
    W0 = np.asarray(W0, f)
    b0 = np.asarray(b0, f)

    scale = f(D) ** f(-0.5)
    # channel o = d*24 + k*8 + m ; column layout is head-major (m, d) -> m*32+d
    md = (np.arange(HEADS)[:, None] + 24 * np.arange(D)[None, :]).reshape(-1)
    q_rows, k_rows, v_rows = md + 0, md + 8, md + 16

    wq_full = np.ascontiguousarray((Wqkv[q_rows, :] * scale).T)   # [256, 256]
    bq_full = (bqkv[q_rows] * scale).reshape(-1, 1)
    wk_full = np.ascontiguousarray(Wqkv[k_rows, :].T)
    bk_full = bqkv[k_rows].reshape(-1, 1)
    w0_full = np.ascontiguousarray(W0.T)                          # [c-in, 256]

    per_mg = []
    for mg in range(2):
        hsl = slice(mg * 128, (mg + 1) * 128)
        # zero-interleaved layouts: per head 32 real rows/cols then 32
        # zeros, so K=64 / M=64 matmuls engage the full PE array
        wq = np.zeros((C, C), f)
        wk = np.zeros((C, C), f)
        bq = np.zeros((C, 1), f)
        bk = np.zeros((C, 1), f)
        wv = np.zeros((C + 1, NV), f)
        for m in range(MG):
            gm = mg * MG + m
            wq[:, m * 64:m * 64 + 32] = wq_full[:, gm * 32:(gm + 1) * 32]
            wk[:, m * 64:m * 64 + 32] = wk_full[:, gm * 32:(gm + 1) * 32]
            bq[m * 64:m * 64 + 32] = bq_full[gm * 32:(gm + 1) * 32]
            bk[m * 64:m * 64 + 32] = bk_full[gm * 32:(gm + 1) * 32]
            vr = v_rows[gm * D:(gm + 1) * D]
            wv[0:C, m * 64:m * 64 + 32] = Wqkv[vr, :].T
            wv[C, m * 64:m * 64 + 32] = bqkv[vr]
            wv[C, m * 64 + 32] = 1.0
        w0b = b0[None, :] if mg == 0 else np.zeros((1, C), f)
        per_mg.append({
            "wq": wq.astype(NPBF),
            "bq": bq,
            "wk": wk.astype(NPBF),
            "bk": bk,
            "wv": wv.astype(NPBF),
            "w0": np.ascontiguousarray(w0_full[hsl, :]).astype(NPBF),
            "w0b": np.ascontiguousarray(w0b).astype(NPBF),
        })

    maps = []
    for b in range(B):
        x_aug = np.concatenate([x[b].reshape(C, N), np.ones((1, N), f)], axis=0)
        x_bf = np.ascontiguousarray(x_aug).astype(NPBF)
        for mg in range(2):
            maps.append({"x": x_bf, **per_mg[mg]})
    return maps


def assemble_output(ys):
    out = np.empty((B, C, N), np.float32)
    for b in range(B):
        np.add(ys[2 * b], ys[2 * b + 1], out=out[b])
    return out.reshape(B, C, HH, WW)


def run(inputs, trace=False):
    nc = _get_program()
    maps = make_in_maps(**inputs)
    res = bass_utils.run_bass_kernel_spmd(
        nc, maps, core_ids=list(range(NCORES)), trace=trace
    )
    ys = [res.results[c]["y"] for c in range(NCORES)]
    return assemble_output(ys), res.exec_time_ns


def kernel(**inputs):
    out, _ = run(inputs, trace=False)
    return out


# revision 18
# speedup vs baseline: 1.2639x; 1.0517x over previous
"""Multi-head self-attention (1x1-conv QKV -> softmax attention -> 1x1-conv)
on Trainium2, 8 NeuronCores, sharded by (batch, head-group).

Problem (hardcoded): x[4,256,48,48], Wqkv[768,256], bqkv[768], W0[256,256],
b0[256]; heads=8, dim_head=32, n=2304 pixels.

Sharding: core = b*2 + mg, mg in {0,1} a group of 4 heads. Each core runs
QKV projection (its 4 heads), attention over all 2304 queries x 2304 keys,
and a PARTIAL output projection y_mg = W0[:, mg-heads] @ out_mg (+ b0 on
mg=0 only). The host sums the two partials per image — a pure elementwise
add during output assembly. No cross-core communication on device.

Per-core dataflow — all large matmuls in BF16 (1 cyc/row on the PE vs 4
for fp32; tolerance is 2e-2 so bf16 is plenty):
  - x_aug [257, 2304] bf16: image + ones row, DMA'd in 512-col chunks so
    projections start early.
  - k,q [(4 heads x 32)=128, 2304] bf16 (Wq, bq pre-scaled by d^-0.5 on
    host), vT [j, 4*(32+1)=132] bf16: per head 32 v-dims + ones col
    (bias + softmax denominator via the x ones-row / vt ones-col tricks).
  - scores^T S_T[j, i] per head pair: K=32 bf16 matmuls row-packed via
    tile_position; each matmul output owns a full PSUM bank. Queries in
    chunks (512,512,512,512,256) — wide chunks keep PE array utilization
    high (narrow ones let the HAM clock-gate re-throttle to 1.2 GHz).
  - P = exp(S_T), split across TWO engines per key-tile j:
      * ACT: table exp (exact), bf16 out
      * DVE: Schraudolph bit-trick exp targeting bf16 bits: one
        scalar_tensor_tensor (st*A16 + B16) -> int16 tile, bitcast bf16
        (max elementwise err ~3.4%; mostly cancels post-softmax).
  - out^T+den: the head PAIR's PV matmuls share ONE PSUM bank: a K=1
    "opener" matmul (start=True) writes a pattern row — 0.0 on the output
    rows 0:33/64:97, 1.0 on the junk rows — so both heads can accumulate
    with start=False (pending-zero bytes make their first write an
    overwrite) and the junk rows stay reciprocal-safe. All MMs of a group
    are chained with chain_iter_dep so the scheduler cannot move an
    accumulate past the closing stop.
  - HAM fillers: the PE clock-gate re-throttles unless the array stays
    busy; full-array K=128 filler matmuls over resident tiles run in the
    queue slots where the PV matmul would otherwise sit waiting, and each
    chunk's normalize is deferred into the next chunk's j-loop so the PE
    never drains at a chunk boundary.
  - normalize: dens sit at psum partitions 32/96. Two 32-row block copies
    to a base-0 tile (custom DVE ops need base 0), one
    reciprocal_approx_fast over 64 partitions, recips staged to bf16,
    then a K=64 bf16 mask matmul broadcasts recipA to partitions 0:32 and
    recipB to 64:96 of an rr PSUM bank; ACT stages rr to SBUF (DVE reads
    at most one PSUM operand) and one [128,w] DVE mul writes normalized
    bf16 outc.
  - y_mg = W0_mg @ outc (+ b0 via ones-row matmul on mg=0), fp32 out,
    DMA per chunk.
"""

import os as _os

import numpy as np
import ml_dtypes

import concourse.bass as bass
import concourse.mybir as mybir
import concourse.tile as tile
from concourse import bacc
from concourse import bass_utils

F32 = mybir.dt.float32
BF = mybir.dt.bfloat16
I16 = mybir.dt.int16
AF = mybir.ActivationFunctionType
ALU = mybir.AluOpType
NPBF = ml_dtypes.bfloat16

B, C, HH, WW = 4, 256, 48, 48
HEADS, D = 8, 32
N = HH * WW            # 2304 pixels = queries = keys per core
NCORES = 8
JT = N // 128          # 18 key tiles
MG = 4                 # heads per core
NV = MG * 64           # 256: vT cols/head: 32 v + ones + 31 zeros (PE-array padding)
ICW = 512              # query chunk tile width

# query chunks: wide and uniform; only one narrow tail (narrow chunks drop
# PE array utilization enough for the HAM clock-gate to re-throttle)
QCHUNKS = [(0, 480), (480, 480), (960, 480), (1440, 480), (1920, 384)]

# Schraudolph exp in bf16 bit space: exp(s) ~ bitcast_bf16(int16(A16*s + B16))
A16_SCHR = float((1 << 7) / np.log(2.0))         # 184.665
B16_SCHR = float((127 << 7) - 5.375)             # C16=5.375: ~3.4% max elem err

N_SPIN0 = int(_os.environ.get("KSPIN0", "16"))   # warmup matmuls before projections
N_FILLN = int(_os.environ.get("KFILL", "2"))     # filler matmuls before each PV pair
N_FILL = int(_os.environ.get("KFILLW", "512"))   # filler matmul free-dim cols

N_DVE = int(_os.environ.get("KDVE", "14"))       # of 2*JT=36 exp instrs on DVE
# keep early slots on ACT: each chunk's DVE queue first drains the previous
# chunk's normalize ops
DVE_SLOTS = frozenset(6 + ((2 * i + 1) * (2 * JT - 6)) // (2 * N_DVE) for i in range(N_DVE))
PV_LAG = 4                                       # pv emission lag (j-iterations)


def _chunks(total, step):
    out = []
    o = 0
    while o < total:
        w = min(step, total - o)
        out.append((o, w))
        o += w
    return out


def _body(tc, x_d, wq_d, bq_d, wk_d, bk_d, wv_d, w0_d, w0b_d, y_d):
    from contextlib import ExitStack

    nc = tc.nc
    with ExitStack() as ctx:
        const = ctx.enter_context(tc.tile_pool(name="const", bufs=1))
        data = ctx.enter_context(tc.tile_pool(name="data", bufs=1))

        # ---------------- load inputs ----------------
        x_sb = [const.tile([128, N], BF, name=f"xa{t}", tag=f"xa{t}") for t in range(2)]
        x1_sb = const.tile([1, N], BF, name="xones", tag="xones")
        for (o, w) in _chunks(N, N // 2):
            nc.sync.dma_start(x_sb[0][:, o:o + w], x_d[0:128, o:o + w])
            nc.sync.dma_start(x_sb[1][:, o:o + w], x_d[128:256, o:o + w])
        nc.gpsimd.dma_start(x1_sb[:], x_d[256:257, :])

        def load2(name, dram, cols, dt=BF):
            ts_ = [const.tile([128, cols], dt, name=f"{name}{t}", tag=f"{name}{t}") for t in range(2)]
            nc.sync.dma_start(ts_[0][:], dram[0:128, :])
            nc.sync.dma_start(ts_[1][:], dram[128:256, :])
            return ts_

        # wq/wk: [256 chan-in, 256 out] — per head 32 real cols + 32 zero
        # cols, so k/q carry zero-interleaved rows and the K=64 score
        # matmuls light the whole PE array (zeros add nothing)
        wq_sb = load2("wq", wq_d, C)
        wk_sb = load2("wk", wk_d, C)
        wv_sb = load2("wv", wv_d, NV)
        wv1_sb = const.tile([1, NV], BF, name="wvbias", tag="wvbias")
        nc.gpsimd.dma_start(wv1_sb[:], wv_d[256:257, :])
        w0_sb = const.tile([128, C], BF, name="w0", tag="w0")
        nc.sync.dma_start(w0_sb[:], w0_d[0:128, :])
        w01_sb = const.tile([1, C], BF, name="w0bias", tag="w0bias")
        nc.gpsimd.dma_start(w01_sb[:], w0b_d[0:1, :])
        bq_sb = load2("bq", bq_d, 1, dt=F32)
        bk_sb = load2("bk", bk_d, 1, dt=F32)

        ones_row = const.tile([1, N], BF, name="ones_row", tag="ones_row")
        nc.vector.memset(ones_row[:], 1.0)
        # Schraudolph additive constant, matching the exp input AP shape
        bexp = const.tile([128, 2 * ICW], F32, name="bexp", tag="bexp")
        nc.vector.memset(bexp[:], B16_SCHR)
        # bank-opener row: 0 over the PV output rows (0:33, 64:97), 1.0 over
        # the junk rows so the den blocks stay reciprocal-safe
        patt = const.tile([1, 128], BF, name="patt", tag="patt")
        nc.vector.memset(patt[:], 1.0)
        nc.vector.memset(patt[0:1, 0:33], 0.0)
        nc.vector.memset(patt[0:1, 64:97], 0.0)

        # persistent activations: pair tile g holds [kA, 0, kB, 0] rows
        k_sb = [data.tile([128, N], BF, name=f"k{g}", tag=f"k{g}") for g in range(2)]
        q_sb = [data.tile([128, N], BF, name=f"q{g}", tag=f"q{g}") for g in range(2)]
        vt_sb = [data.tile([128, NV], BF, name=f"vt{j}", tag=f"vt{j}") for j in range(JT)]
        # output tiles in pv layout: tile pr holds head 2*pr at partitions
        # 0-31 and head 2*pr+1 at partitions 64-95
        outc_sb = [data.tile([128, N], BF, name=f"oc{t}", tag=f"oc{t}") for t in range(2)]
        y_sb = [data.tile([128, N], F32, name=f"y{g}", tag=f"y{g}") for g in range(2)]

        # ---------------- projections (bf16) ----------------
        with tc.tile_pool(name="prj", bufs=2, space="PSUM") as prj:
            # warm the PE during the x-DMA wait so projections start at
            # full clock: full-array matmuls into a scratch psum tile
            wt = prj.tile([128, 512], F32, name="wt", tag="wt")
            for i in range(N_SPIN0):
                o = 256 * (i & 1)
                nc.tensor.matmul(
                    wt[:, o:o + 256], wq_sb[0][:, 0:128], x_sb[0][:, 0:256],
                    start=True, stop=True, tile_position=(0, 0),
                )
            # interleave the dense 512-wide k/q chunks between the narrow
            # V-projection tiles so PE array utilization never dips long
            # enough for the HAM clock-gate to re-throttle
            def emit_kq(g, o, w):
                gsl = slice(g * 128, (g + 1) * 128)
                kps = prj.tile([128, 512], F32, name="kps", tag="kps")
                nc.tensor.matmul(kps[:, :w], wk_sb[0][:, gsl], x_sb[0][:, o:o + w], start=True, stop=False)
                nc.tensor.matmul(kps[:, :w], wk_sb[1][:, gsl], x_sb[1][:, o:o + w], start=False, stop=True)
                nc.scalar.activation(k_sb[g][:, o:o + w], kps[:, :w], AF.Identity, bias=bk_sb[g][:, 0:1])
                qps = prj.tile([128, 512], F32, name="qps", tag="qps")
                nc.tensor.matmul(qps[:, :w], wq_sb[0][:, gsl], x_sb[0][:, o:o + w], start=True, stop=False)
                nc.tensor.matmul(qps[:, :w], wq_sb[1][:, gsl], x_sb[1][:, o:o + w], start=False, stop=True)
                nc.scalar.activation(q_sb[g][:, o:o + w], qps[:, :w], AF.Identity, bias=bq_sb[g][:, 0:1])

            def emit_v(j):
                jsl = slice(j * 128, (j + 1) * 128)
                vps = prj.tile([128, NV], F32, name="vps", tag="vps")
                nc.tensor.matmul(vps[:], x_sb[0][:, jsl], wv_sb[0][:], start=True, stop=False)
                nc.tensor.matmul(vps[:], x_sb[1][:, jsl], wv_sb[1][:], start=False, stop=False)
                nc.tensor.matmul(vps[:], x1_sb[:, jsl], wv1_sb[:], start=False, stop=True)
                if j % 2 == 0:
                    nc.scalar.copy(vt_sb[j][:], vps[:])
                else:
                    nc.vector.tensor_copy(vt_sb[j][:], vps[:])

            kq_units = [(g, o, w) for g in range(2) for (o, w) in _chunks(N, 512)]
            vj = 0
            for i, (g, o, w) in enumerate(kq_units):
                emit_kq(g, o, w)
                while vj < JT and vj < 2 * (i + 1):
                    emit_v(vj)
                    vj += 1
            while vj < JT:
                emit_v(vj)
                vj += 1

        # ---------------- attention main loop ----------------
        # Both head pairs interleave in one j-loop: the other pair's
        # matmuls fill the PE queue slots where a lone pair would sit
        # waiting on exp, so the PE stays busy with real work and the HAM
        # clock-gate keeps the array at 2.4 GHz. Both exp engines (ACT
        # table exp / DVE Schraudolph) run every j.
        # PSUM: stp 3x2 banks (scores; rr matmuls borrow ring slots)
        # + pvp 2x1 (bank-shared pair each) = 8.
        with tc.tile_pool(name="stp", bufs=3, space="PSUM") as stp, \
             tc.tile_pool(name="pvp", bufs=1, space="PSUM") as pvp, \
             tc.tile_pool(name="ptp", bufs=12) as ptp, \
             tc.tile_pool(name="epi", bufs=4) as epi:

            pending = []   # deferred normalize muls: list of closures

            def flush_norm():
                while pending:
                    pending.pop(0)()

            for (ic0, w) in QCHUNKS:
                pvs = [pvp.tile([128, ICW], F32, name=f"pv{p}", tag=f"pv{p}") for p in range(2)]
                pts = {}

                def emit_pv(j, w=w, pvs=pvs, pts=pts, ic0=ic0):
                    for p in range(2):
                        pt = pts.pop((j, p))
                        for hl, base in ((0, 0), (1, 64)):
                            gh = 2 * p + hl
                            mi = nc.tensor.matmul(
                                pvs[p][base:base + 64, 0:w],
                                vt_sb[j][:, gh * 64:gh * 64 + 64],
                                pt[:].bitcast(BF)[:, hl * ICW:hl * ICW + w],
                                start=False,
                                stop=(j == JT - 1 and hl == 1),
                                tile_position=(0, base),
                            )
                            tc.chain_iter_dep(f"pvc{p}_{ic0}", mi.ins)

                for j in range(JT):
                    for p in range(2):
                        st = stp.tile([128, 1024], F32, name="st", tag="st")
                        for hl in range(2):
                            nc.tensor.matmul(
                                st[:, hl * 512:hl * 512 + w],
                                k_sb[p][hl * 64:(hl + 1) * 64, j * 128:(j + 1) * 128],
                                q_sb[p][hl * 64:(hl + 1) * 64, ic0:ic0 + w],
                                start=True, stop=True,
                                tile_position=(hl * 64, 0),
                            )
                        st_v = st[:].rearrange("p (s q) -> p s q", s=2)[:, :, 0:w]
                        if (2 * j + p) in DVE_SLOTS:
                            pt = ptp.tile([128, 2 * ICW], I16, name="pt", tag="pt")
                            nc.vector.scalar_tensor_tensor(
                                pt[:].rearrange("p (s q) -> p s q", s=2)[:, :, 0:w],
                                st_v, A16_SCHR,
                                bexp[:].rearrange("p (s q) -> p s q", s=2)[:, :, 0:w],
                                ALU.mult, ALU.add,
                            )
                        else:
                            pt = ptp.tile([128, 2 * ICW], BF, name="pt", tag="pt")
                            nc.scalar.activation(
                                pt[:].rearrange("p (s q) -> p s q", s=2)[:, :, 0:w],
                                st_v, AF.Exp,
                            )
                        pts[(j, p)] = pt
                    if j == 1:
                        # previous chunk's deferred normalize: the j0/j1
                        # score sets ahead of the rr matmuls cover the DVE
                        # recip chain latency
                        flush_norm()
                    if j == 3:
                        # open the shared banks: K=1 matmuls write the
                        # pattern row to all 128 partitions, start=True.
                        # Late enough that the pool-release wait (previous
                        # chunk's normalize muls) is already satisfied.
                        for p in range(2):
                            mi = nc.tensor.matmul(
                                pvs[p][:, 0:w], patt[0:1, 0:128], ones_row[0:1, 0:w],
                                start=True, stop=False, tile_position=(0, 0),
                            )
                            tc.chain_iter_dep(f"pvc{p}_{ic0}", mi.ins)
                    if j >= PV_LAG:
                        emit_pv(j - PV_LAG)
                for j in range(JT - PV_LAG, JT):
                    emit_pv(j)

                # normalize (all DVE/no PSUM pressure): dens at psum
                # partitions 32/96, junk rows hold 1.0 from the opener (PV
                # zero-padding accumulates zeros onto them). Block copies
                # to a base-0 tile (custom DVE ops need base 0), one
                # reciprocal over 64 partitions, then intra-block
                # stream_shuffle broadcasts + two muls per pair. The muls
                # (which free the pv banks) are deferred into the next
                # chunk so the j-loop tail keeps both engines busy.
                for p in range(2):
                    pv = pvs[p]
                    oc = outc_sb[p]
                    dd = epi.tile([64, ICW], F32, name="dd", tag="dd")
                    rc = epi.tile([64, ICW], F32, name="rc", tag="rc")
                    rba = epi.tile([128, ICW], F32, name="rba", tag="rba")
                    nc.vector.tensor_copy(dd[0:32, 0:w], pv[32:64, 0:w])
                    nc.vector.tensor_copy(dd[32:64, 0:w], pv[96:128, 0:w])
                    nc.vector.reciprocal_approx_fast(rc[0:64, 0:w], dd[0:64, 0:w])
                    nc.vector.stream_shuffle(rba[0:32, 0:w], rc[0:32, 0:w], [0] * 32)
                    nc.vector.stream_shuffle(rba[32:64, 0:w], rc[32:64, 0:w], [0] * 32)
                    nc.vector.tensor_copy(rba[64:96, 0:w], rba[32:64, 0:w])

                    def mul_norm(pv=pv, oc=oc, rba=rba, ic0=ic0, w=w):
                        nc.vector.tensor_mul(oc[0:33, ic0:ic0 + w], pv[0:33, 0:w], rba[0:33, 0:w])
                        nc.vector.tensor_mul(oc[64:97, ic0:ic0 + w], pv[64:97, 0:w], rba[64:97, 0:w])
                    pending.append(mul_norm)
            flush_norm()

        # ---------------- partial output projection ----------------
        # compact pv-layout outc tiles into one dense head-major [128, i]
        # tile via SBUF->SBUF DMA partition remap, then K=128 matmuls
        # against this head-group's 128 rows of W0. Bias comes via the
        # ones-row matmul (host zeroes it for mg=1 so it is added once).
        od_sb = data.tile([128, N], BF, name="od", tag="od")
        for pr in range(2):
            src = outc_sb[pr]
            nc.sync.dma_start(od_sb[pr * 64:pr * 64 + 32, :], src[0:32, :])
            nc.sync.dma_start(od_sb[pr * 64 + 32:pr * 64 + 64, :], src[64:96, :])
        with tc.tile_pool(name="fin", bufs=2, space="PSUM") as fin:
            for mt in range(2):
                msl = slice(mt * 128, (mt + 1) * 128)
                for (o, w) in _chunks(N, 512):
                    fps = fin.tile([128, 512], F32, name="fps", tag="fps")
                    nc.tensor.matmul(fps[:, :w], w0_sb[:, msl], od_sb[:, o:o + w], start=True, stop=False)
                    nc.tensor.matmul(fps[:, :w], w01_sb[:, msl], ones_row[:, o:o + w], start=False, stop=True)
                    nc.scalar.copy(y_sb[mt][:, o:o + w], fps[:, :w])
                    nc.sync.dma_start(y_d[msl, o:o + w], y_sb[mt][:, o:o + w])


def build_program():
    nc = bacc.Bacc(
        "TRN2",
        target_bir_lowering=False,
        debug=False,
        enable_asserts=False,
        num_devices=NCORES,
    )
    x_d = nc.dram_tensor("x", [C + 1, N], BF, kind="ExternalInput").ap()
    wq_d = nc.dram_tensor("wq", [C, C], BF, kind="ExternalInput").ap()
    bq_d = nc.dram_tensor("bq", [C, 1], F32, kind="ExternalInput").ap()
    wk_d = nc.dram_tensor("wk", [C, C], BF, kind="ExternalInput").ap()
    bk_d = nc.dram_tensor("bk", [C, 1], F32, kind="ExternalInput").ap()
    wv_d = nc.dram_tensor("wv", [C + 1, NV], BF, kind="ExternalInput").ap()
    w0_d = nc.dram_tensor("w0", [128, C], BF, kind="ExternalInput").ap()
    w0b_d = nc.dram_tensor("w0b", [1, C], BF, kind="ExternalInput").ap()
    y_d = nc.dram_tensor("y", [C, N], F32, kind="ExternalOutput").ap()

    with tile.TileContext(nc) as tc:
        _body(tc, x_d, wq_d, bq_d, wk_d, bk_d, wv_d, w0_d, w0b_d, y_d)
    nc.compile()
    return nc


_CACHE = {}


def _get_program():
    if "nc" not in _CACHE:
        _CACHE["nc"] = build_program()
    return _CACHE["nc"]


def make_in_maps(x, Wqkv, bqkv, W0, b0):
    f = np.float32
    x = np.asarray(x, f)
    Wqkv = np.asarray(Wqkv, f)
    bqkv = np.asarray(bqkv, f)
    W0 = np.asarray(W0, f)
    b0 = np.asarray(b0, f)

    scale = f(D) ** f(-0.5)
    # channel o = d*24 + k*8 + m ; column layout is head-major (m, d) -> m*32+d
    md = (np.arange(HEADS)[:, None] + 24 * np.arange(D)[None, :]).reshape(-1)
    q_rows, k_rows, v_rows = md + 0, md + 8, md + 16

    wq_full = np.ascontiguousarray((Wqkv[q_rows, :] * scale).T)   # [256, 256]
    bq_full = (bqkv[q_rows] * scale).reshape(-1, 1)
    wk_full = np.ascontiguousarray(Wqkv[k_rows, :].T)
    bk_full = bqkv[k_rows].reshape(-1, 1)
    w0_full = np.ascontiguousarray(W0.T)                          # [c-in, 256]

    per_mg = []
    for mg in range(2):
        hsl = slice(mg * 128, (mg + 1) * 128)
        # zero-interleaved layouts: per head 32 real rows/cols then 32
        # zeros, so K=64 / M=64 matmuls engage the full PE array
        wq = np.zeros((C, C), f)
        wk = np.zeros((C, C), f)
        bq = np.zeros((C, 1), f)
        bk = np.zeros((C, 1), f)
        wv = np.zeros((C + 1, NV), f)
        for m in range(MG):
            gm = mg * MG + m
            wq[:, m * 64:m * 64 + 32] = wq_full[:, gm * 32:(gm + 1) * 32]
            wk[:, m * 64:m * 64 + 32] = wk_full[:, gm * 32:(gm + 1) * 32]
            bq[m * 64:m * 64 + 32] = bq_full[gm * 32:(gm + 1) * 32]
            bk[m * 64:m * 64 + 32] = bk_full[gm * 32:(gm + 1) * 32]
            vr = v_rows[gm * D:(gm + 1) * D]
            wv[0:C, m * 64:m * 64 + 32] = Wqkv[vr, :].T
            wv[C, m * 64:m * 64 + 32] = bqkv[vr]
            wv[C, m * 64 + 32] = 1.0
        w0b = b0[None, :] if mg == 0 else np.zeros((1, C), f)
        per_mg.append({
            "wq": wq.astype(NPBF),
            "bq": bq,
            "wk": wk.astype(NPBF),
            "bk": bk,
            "wv": wv.astype(NPBF),
            "w0": np.ascontiguousarray(w0_full[hsl, :]).astype(NPBF),
            "w0b": np.ascontiguousarray(w0b).astype(NPBF),
        })

    maps = []
    for b in range(B):
        x_aug = np.concatenate([x[b].reshape(C, N), np.ones((1, N), f)], axis=0)
        x_bf = np.ascontiguousarray(x_aug).astype(NPBF)
        for mg in range(2):
            maps.append({"x": x_bf, **per_mg[mg]})
    return maps


def assemble_output(ys):
    out = np.empty((B, C, N), np.float32)
    for b in range(B):
        np.add(ys[2 * b], ys[2 * b + 1], out=out[b])
    return out.reshape(B, C, HH, WW)


def run(inputs, trace=False):
    nc = _get_program()
    maps = make_in_maps(**inputs)
    res = bass_utils.run_bass_kernel_spmd(
        nc, maps, core_ids=list(range(NCORES)), trace=trace
    )
    ys = [res.results[c]["y"] for c in range(NCORES)]
    return assemble_output(ys), res.exec_time_ns


def kernel(**inputs):
    out, _ = run(inputs, trace=False)
    return out


# revision 19
# speedup vs baseline: 1.2753x; 1.0090x over previous
"""Multi-head self-attention (1x1-conv QKV -> softmax attention -> 1x1-conv)
on Trainium2, 8 NeuronCores, sharded by (batch, head-group).

Problem (hardcoded): x[4,256,48,48], Wqkv[768,256], bqkv[768], W0[256,256],
b0[256]; heads=8, dim_head=32, n=2304 pixels.

Sharding: core = b*2 + mg, mg in {0,1} a group of 4 heads. Each core runs
QKV projection (its 4 heads), attention over all 2304 queries x 2304 keys,
and a PARTIAL output projection y_mg = W0[:, mg-heads] @ out_mg (+ b0 on
mg=0 only). The host sums the two partials per image — a pure elementwise
add during output assembly. No cross-core communication on device.

Per-core dataflow — all large matmuls in BF16 (1 cyc/row on the PE vs 4
for fp32; tolerance is 2e-2 so bf16 is plenty):
  - x_aug [257, 2304] bf16: image + ones row, DMA'd in 512-col chunks so
    projections start early.
  - k,q [(4 heads x 32)=128, 2304] bf16 (Wq, bq pre-scaled by d^-0.5 on
    host), vT [j, 4*(32+1)=132] bf16: per head 32 v-dims + ones col
    (bias + softmax denominator via the x ones-row / vt ones-col tricks).
  - scores^T S_T[j, i] per head pair: K=32 bf16 matmuls row-packed via
    tile_position; each matmul output owns a full PSUM bank. Queries in
    chunks (512,512,512,512,256) — wide chunks keep PE array utilization
    high (narrow ones let the HAM clock-gate re-throttle to 1.2 GHz).
  - P = exp(S_T), split across TWO engines per key-tile j:
      * ACT: table exp (exact), bf16 out
      * DVE: Schraudolph bit-trick exp targeting bf16 bits: one
        scalar_tensor_tensor (st*A16 + B16) -> int16 tile, bitcast bf16
        (max elementwise err ~3.4%; mostly cancels post-softmax).
  - out^T+den: the head PAIR's PV matmuls share ONE PSUM bank: a K=1
    "opener" matmul (start=True) writes a pattern row — 0.0 on the output
    rows 0:33/64:97, 1.0 on the junk rows — so both heads can accumulate
    with start=False (pending-zero bytes make their first write an
    overwrite) and the junk rows stay reciprocal-safe. All MMs of a group
    are chained with chain_iter_dep so the scheduler cannot move an
    accumulate past the closing stop.
  - HAM fillers: the PE clock-gate re-throttles unless the array stays
    busy; full-array K=128 filler matmuls over resident tiles run in the
    queue slots where the PV matmul would otherwise sit waiting, and each
    chunk's normalize is deferred into the next chunk's j-loop so the PE
    never drains at a chunk boundary.
  - normalize: dens sit at psum partitions 32/96. Two 32-row block copies
    to a base-0 tile (custom DVE ops need base 0), one
    reciprocal_approx_fast over 64 partitions, recips staged to bf16,
    then a K=64 bf16 mask matmul broadcasts recipA to partitions 0:32 and
    recipB to 64:96 of an rr PSUM bank; ACT stages rr to SBUF (DVE reads
    at most one PSUM operand) and one [128,w] DVE mul writes normalized
    bf16 outc.
  - y_mg = W0_mg @ outc (+ b0 via ones-row matmul on mg=0), fp32 out,
    DMA per chunk.
"""

import os as _os

import numpy as np
import ml_dtypes

import concourse.bass as bass
import concourse.mybir as mybir
import concourse.tile as tile
from concourse import bacc
from concourse import bass_utils

F32 = mybir.dt.float32
BF = mybir.dt.bfloat16
I16 = mybir.dt.int16
AF = mybir.ActivationFunctionType
ALU = mybir.AluOpType
NPBF = ml_dtypes.bfloat16

B, C, HH, WW = 4, 256, 48, 48
HEADS, D = 8, 32
N = HH * WW            # 2304 pixels = queries = keys per core
NCORES = 8
JT = N // 128          # 18 key tiles
MG = 4                 # heads per core
NV = MG * 64           # 256: vT cols/head: 32 v + ones + 31 zeros (PE-array padding)
ICW = 512              # query chunk tile width

# query chunks: wide and uniform; only one narrow tail (narrow chunks drop
# PE array utilization enough for the HAM clock-gate to re-throttle)
QCHUNKS = [(0, 480), (480, 480), (960, 480), (1440, 480), (1920, 384)]

# Schraudolph exp in bf16 bit space: exp(s) ~ bitcast_bf16(int16(A16*s + B16))
A16_SCHR = float((1 << 7) / np.log(2.0))         # 184.665
B16_SCHR = float((127 << 7) - 5.375)             # C16=5.375: ~3.4% max elem err

N_SPIN0 = int(_os.environ.get("KSPIN0", "16"))   # warmup matmuls before projections
N_FILLN = int(_os.environ.get("KFILL", "2"))     # filler matmuls before each PV pair
N_FILL = int(_os.environ.get("KFILLW", "512"))   # filler matmul free-dim cols

N_DVE = int(_os.environ.get("KDVE", "14"))       # of 2*JT=36 exp instrs on DVE
# keep early slots on ACT: each chunk's DVE queue first drains the previous
# chunk's normalize ops
DVE_SLOTS = frozenset(6 + ((2 * i + 1) * (2 * JT - 6)) // (2 * N_DVE) for i in range(N_DVE))
PV_LAG = 4                                       # pv emission lag (j-iterations)


def _chunks(total, step):
    out = []
    o = 0
    while o < total:
        w = min(step, total - o)
        out.append((o, w))
        o += w
    return out


def _body(tc, x_d, wq_d, bq_d, wk_d, bk_d, wv_d, w0_d, w0b_d, y_d):
    from contextlib import ExitStack

    nc = tc.nc
    with ExitStack() as ctx:
        const = ctx.enter_context(tc.tile_pool(name="const", bufs=1))
        data = ctx.enter_context(tc.tile_pool(name="data", bufs=1))

        # ---------------- load inputs ----------------
        x_sb = [const.tile([128, N], BF, name=f"xa{t}", tag=f"xa{t}") for t in range(2)]
        x1_sb = const.tile([1, N], BF, name="xones", tag="xones")
        for (o, w) in _chunks(N, N // 2):
            nc.sync.dma_start(x_sb[0][:, o:o + w], x_d[0:128, o:o + w])
            nc.sync.dma_start(x_sb[1][:, o:o + w], x_d[128:256, o:o + w])
        nc.gpsimd.dma_start(x1_sb[:], x_d[256:257, :])

        def load2(name, dram, cols, dt=BF):
            ts_ = [const.tile([128, cols], dt, name=f"{name}{t}", tag=f"{name}{t}") for t in range(2)]
            nc.sync.dma_start(ts_[0][:], dram[0:128, :])
            nc.sync.dma_start(ts_[1][:], dram[128:256, :])
            return ts_

        # wq/wk: [256 chan-in, 256 out] — per head 32 real cols + 32 zero
        # cols, so k/q carry zero-interleaved rows and the K=64 score
        # matmuls light the whole PE array (zeros add nothing)
        wq_sb = load2("wq", wq_d, C)
        wk_sb = load2("wk", wk_d, C)
        wv_sb = load2("wv", wv_d, NV)
        wv1_sb = const.tile([1, NV], BF, name="wvbias", tag="wvbias")
        nc.gpsimd.dma_start(wv1_sb[:], wv_d[256:257, :])
        w0_sb = const.tile([128, C], BF, name="w0", tag="w0")
        nc.sync.dma_start(w0_sb[:], w0_d[0:128, :])
        w01_sb = const.tile([1, C], BF, name="w0bias", tag="w0bias")
        nc.gpsimd.dma_start(w01_sb[:], w0b_d[0:1, :])
        bq_sb = load2("bq", bq_d, 1, dt=F32)
        bk_sb = load2("bk", bk_d, 1, dt=F32)

        ones_row = const.tile([1, N], BF, name="ones_row", tag="ones_row")
        nc.vector.memset(ones_row[:], 1.0)
        # Schraudolph additive constant, matching the exp input AP shape
        bexp = const.tile([128, 2 * ICW], F32, name="bexp", tag="bexp")
        nc.vector.memset(bexp[:], B16_SCHR)
        # bank-opener row: 0 over the PV output rows (0:33, 64:97), 1.0 over
        # the junk rows so the den blocks stay reciprocal-safe
        patt = const.tile([1, 128], BF, name="patt", tag="patt")
        nc.vector.memset(patt[:], 1.0)
        nc.vector.memset(patt[0:1, 0:33], 0.0)
        nc.vector.memset(patt[0:1, 64:97], 0.0)

        # persistent activations: pair tile g holds [kA, 0, kB, 0] rows
        k_sb = [data.tile([128, N], BF, name=f"k{g}", tag=f"k{g}") for g in range(2)]
        q_sb = [data.tile([128, N], BF, name=f"q{g}", tag=f"q{g}") for g in range(2)]
        vt_sb = [data.tile([128, NV], BF, name=f"vt{j}", tag=f"vt{j}") for j in range(JT)]
        # output tiles in pv layout: tile pr holds head 2*pr at partitions
        # 0-31 and head 2*pr+1 at partitions 64-95
        outc_sb = [data.tile([128, N], BF, name=f"oc{t}", tag=f"oc{t}") for t in range(2)]
        # out-projection staging: pv-layout outc remapped head-major via
        # SBUF->SBUF DMA, partial y = W0_mg @ od (+ b0 via ones-row on mg=0)
        od_sb = data.tile([128, N], BF, name="od", tag="od")
        y_sb = [data.tile([128, N], F32, name=f"y{g}", tag=f"y{g}") for g in range(2)]

        # ---------------- projections (bf16) ----------------
        with tc.tile_pool(name="prj", bufs=2, space="PSUM") as prj:
            # warm the PE during the x-DMA wait so projections start at
            # full clock: full-array matmuls into a scratch psum tile
            wt = prj.tile([128, 512], F32, name="wt", tag="wt")
            for i in range(N_SPIN0):
                o = 256 * (i & 1)
                nc.tensor.matmul(
                    wt[:, o:o + 256], wq_sb[0][:, 0:128], x_sb[0][:, 0:256],
                    start=True, stop=True, tile_position=(0, 0),
                )
            # interleave the dense 512-wide k/q chunks between the narrow
            # V-projection tiles so PE array utilization never dips long
            # enough for the HAM clock-gate to re-throttle
            def emit_kq(g, o, w):
                gsl = slice(g * 128, (g + 1) * 128)
                kps = prj.tile([128, 512], F32, name="kps", tag="kps")
                nc.tensor.matmul(kps[:, :w], wk_sb[0][:, gsl], x_sb[0][:, o:o + w], start=True, stop=False)
                nc.tensor.matmul(kps[:, :w], wk_sb[1][:, gsl], x_sb[1][:, o:o + w], start=False, stop=True)
                nc.scalar.activation(k_sb[g][:, o:o + w], kps[:, :w], AF.Identity, bias=bk_sb[g][:, 0:1])
                qps = prj.tile([128, 512], F32, name="qps", tag="qps")
                nc.tensor.matmul(qps[:, :w], wq_sb[0][:, gsl], x_sb[0][:, o:o + w], start=True, stop=False)
                nc.tensor.matmul(qps[:, :w], wq_sb[1][:, gsl], x_sb[1][:, o:o + w], start=False, stop=True)
                nc.scalar.activation(q_sb[g][:, o:o + w], qps[:, :w], AF.Identity, bias=bq_sb[g][:, 0:1])

            def emit_v(j):
                jsl = slice(j * 128, (j + 1) * 128)
                vps = prj.tile([128, NV], F32, name="vps", tag="vps")
                nc.tensor.matmul(vps[:], x_sb[0][:, jsl], wv_sb[0][:], start=True, stop=False)
                nc.tensor.matmul(vps[:], x_sb[1][:, jsl], wv_sb[1][:], start=False, stop=False)
                nc.tensor.matmul(vps[:], x1_sb[:, jsl], wv1_sb[:], start=False, stop=True)
                if j % 2 == 0:
                    nc.scalar.copy(vt_sb[j][:], vps[:])
                else:
                    nc.vector.tensor_copy(vt_sb[j][:], vps[:])

            kq_units = [(g, o, w) for g in range(2) for (o, w) in _chunks(N, 512)]
            vj = 0
            for i, (g, o, w) in enumerate(kq_units):
                emit_kq(g, o, w)
                while vj < JT and vj < 2 * (i + 1):
                    emit_v(vj)
                    vj += 1
            while vj < JT:
                emit_v(vj)
                vj += 1

        # ---------------- attention main loop ----------------
        # Both head pairs interleave in one j-loop: the other pair's
        # matmuls fill the PE queue slots where a lone pair would sit
        # waiting on exp, so the PE stays busy with real work and the HAM
        # clock-gate keeps the array at 2.4 GHz. Both exp engines (ACT
        # table exp / DVE Schraudolph) run every j.
        # PSUM: stp 3x2 banks (scores; rr matmuls borrow ring slots)
        # + pvp 2x1 (bank-shared pair each) = 8.
        with tc.tile_pool(name="stp", bufs=3, space="PSUM") as stp, \
             tc.tile_pool(name="pvp", bufs=1, space="PSUM") as pvp, \
             tc.tile_pool(name="ptp", bufs=12) as ptp, \
             tc.tile_pool(name="epi", bufs=4) as epi:

            pending = []   # deferred normalize muls: list of closures
            pending_dma = []   # od-remap DMAs for normalized chunks
            pending_fin = []   # out-projection matmuls for remapped chunks

            def flush_norm():
                while pending:
                    pending.pop(0)()

            def emit_od_dma(ic0, w):
                for p in range(2):
                    src_t = outc_sb[p]
                    nc.sync.dma_start(od_sb[p * 64:p * 64 + 32, ic0:ic0 + w], src_t[0:32, ic0:ic0 + w])
                    nc.sync.dma_start(od_sb[p * 64 + 32:p * 64 + 64, ic0:ic0 + w], src_t[64:96, ic0:ic0 + w])

            def emit_fin(ic0, w):
                for mt in range(2):
                    msl = slice(mt * 128, (mt + 1) * 128)
                    fps = stp.tile([128, 1024], F32, name="fps", tag="st")
                    nc.tensor.matmul(fps[:, :w], w0_sb[:, msl], od_sb[:, ic0:ic0 + w], start=True, stop=False)
                    nc.tensor.matmul(fps[:, :w], w01_sb[:, msl], ones_row[:, ic0:ic0 + w], start=False, stop=True)
                    nc.scalar.copy(y_sb[mt][:, ic0:ic0 + w], fps[:, :w])
                    nc.sync.dma_start(y_d[msl, ic0:ic0 + w], y_sb[mt][:, ic0:ic0 + w])

            for (ic0, w) in QCHUNKS:
                pvs = [pvp.tile([128, ICW], F32, name=f"pv{p}", tag=f"pv{p}") for p in range(2)]
                pts = {}

                def emit_pv(j, w=w, pvs=pvs, pts=pts, ic0=ic0):
                    for p in range(2):
                        pt = pts.pop((j, p))
                        for hl, base in ((0, 0), (1, 64)):
                            gh = 2 * p + hl
                            mi = nc.tensor.matmul(
                                pvs[p][base:base + 64, 0:w],
                                vt_sb[j][:, gh * 64:gh * 64 + 64],
                                pt[:].bitcast(BF)[:, hl * ICW:hl * ICW + w],
                                start=False,
                                stop=(j == JT - 1 and hl == 1),
                                tile_position=(0, base),
                            )
                            tc.chain_iter_dep(f"pvc{p}_{ic0}", mi.ins)

                for j in range(JT):
                    for p in range(2):
                        st = stp.tile([128, 1024], F32, name="st", tag="st")
                        for hl in range(2):
                            nc.tensor.matmul(
                                st[:, hl * 512:hl * 512 + w],
                                k_sb[p][hl * 64:(hl + 1) * 64, j * 128:(j + 1) * 128],
                                q_sb[p][hl * 64:(hl + 1) * 64, ic0:ic0 + w],
                                start=True, stop=True,
                                tile_position=(hl * 64, 0),
                            )
                        st_v = st[:].rearrange("p (s q) -> p s q", s=2)[:, :, 0:w]
                        if (2 * j + p) in DVE_SLOTS:
                            pt = ptp.tile([128, 2 * ICW], I16, name="pt", tag="pt")
                            nc.vector.scalar_tensor_tensor(
                                pt[:].rearrange("p (s q) -> p s q", s=2)[:, :, 0:w],
                                st_v, A16_SCHR,
                                bexp[:].rearrange("p (s q) -> p s q", s=2)[:, :, 0:w],
                                ALU.mult, ALU.add,
                            )
                        else:
                            pt = ptp.tile([128, 2 * ICW], BF, name="pt", tag="pt")
                            nc.scalar.activation(
                                pt[:].rearrange("p (s q) -> p s q", s=2)[:, :, 0:w],
                                st_v, AF.Exp,
                            )
                        pts[(j, p)] = pt
                    if j == 1:
                        # previous chunk's deferred normalize muls
                        flush_norm()
                    if j == 4 and pending_dma:
                        emit_od_dma(*pending_dma.pop(0))
                    if j == 12 and pending_fin:
                        emit_fin(*pending_fin.pop(0))
                    if j == 3:
                        # open the shared banks: K=1 matmuls write the
                        # pattern row to all 128 partitions, start=True.
                        # Late enough that the pool-release wait (previous
                        # chunk's normalize muls) is already satisfied.
                        for p in range(2):
                            mi = nc.tensor.matmul(
                                pvs[p][:, 0:w], patt[0:1, 0:128], ones_row[0:1, 0:w],
                                start=True, stop=False, tile_position=(0, 0),
                            )
                            tc.chain_iter_dep(f"pvc{p}_{ic0}", mi.ins)
                    if j >= PV_LAG:
                        emit_pv(j - PV_LAG)
                for j in range(JT - PV_LAG, JT):
                    emit_pv(j)

                # normalize (all DVE/no PSUM pressure): dens at psum
                # partitions 32/96, junk rows hold 1.0 from the opener (PV
                # zero-padding accumulates zeros onto them). Block copies
                # to a base-0 tile (custom DVE ops need base 0), one
                # reciprocal over 64 partitions, then intra-block
                # stream_shuffle broadcasts + two muls per pair. The muls
                # (which free the pv banks) are deferred into the next
                # chunk so the j-loop tail keeps both engines busy.
                for p in range(2):
                    pv = pvs[p]
                    oc = outc_sb[p]
                    dd = epi.tile([64, ICW], F32, name="dd", tag="dd")
                    rc = epi.tile([64, ICW], F32, name="rc", tag="rc")
                    rba = epi.tile([128, ICW], F32, name="rba", tag="rba")
                    nc.vector.tensor_copy(dd[0:32, 0:w], pv[32:64, 0:w])
                    nc.vector.tensor_copy(dd[32:64, 0:w], pv[96:128, 0:w])
                    nc.vector.reciprocal_approx_fast(rc[0:64, 0:w], dd[0:64, 0:w])
                    nc.vector.stream_shuffle(rba[0:32, 0:w], rc[0:32, 0:w], [0] * 32)
                    nc.vector.stream_shuffle(rba[32:64, 0:w], rc[32:64, 0:w], [0] * 32)
                    nc.vector.tensor_copy(rba[64:96, 0:w], rba[32:64, 0:w])

                    def mul_norm(pv=pv, oc=oc, rba=rba, ic0=ic0, w=w):
                        nc.vector.tensor_mul(oc[0:33, ic0:ic0 + w], pv[0:33, 0:w], rba[0:33, 0:w])
                        nc.vector.tensor_mul(oc[64:97, ic0:ic0 + w], pv[64:97, 0:w], rba[64:97, 0:w])
                    pending.append(mul_norm)
                pending_dma.append((ic0, w))
                pending_fin.append((ic0, w))
            flush_norm()
            while pending_dma:
                emit_od_dma(*pending_dma.pop(0))
            while pending_fin:
                emit_fin(*pending_fin.pop(0))



def build_program():
    nc = bacc.Bacc(
        "TRN2",
        target_bir_lowering=False,
        debug=False,
        enable_asserts=False,
        num_devices=NCORES,
    )
    x_d = nc.dram_tensor("x", [C + 1, N], BF, kind="ExternalInput").ap()
    wq_d = nc.dram_tensor("wq", [C, C], BF, kind="ExternalInput").ap()
    bq_d = nc.dram_tensor("bq", [C, 1], F32, kind="ExternalInput").ap()
    wk_d = nc.dram_tensor("wk", [C, C], BF, kind="ExternalInput").ap()
    bk_d = nc.dram_tensor("bk", [C, 1], F32, kind="ExternalInput").ap()
    wv_d = nc.dram_tensor("wv", [C + 1, NV], BF, kind="ExternalInput").ap()
    w0_d = nc.dram_tensor("w0", [128, C], BF, kind="ExternalInput").ap()
    w0b_d = nc.dram_tensor("w0b", [1, C], BF, kind="ExternalInput").ap()
    y_d = nc.dram_tensor("y", [C, N], F32, kind="ExternalOutput").ap()

    with tile.TileContext(nc) as tc:
        _body(tc, x_d, wq_d, bq_d, wk_d, bk_d, wv_d, w0_d, w0b_d, y_d)
    nc.compile()
    return nc


_CACHE = {}


def _get_program():
    if "nc" not in _CACHE:
        _CACHE["nc"] = build_program()
    return _CACHE["nc"]


def make_in_maps(x, Wqkv, bqkv, W0, b0):
    f = np.float32
    x = np.asarray(x, f)
    Wqkv = np.asarray(Wqkv, f)
    bqkv = np.asarray(bqkv, f)
    W0 = np.asarray(W0, f)
    b0 = np.asarray(b0, f)

    scale = f(D) ** f(-0.5)
    # channel o = d*24 + k*8 + m ; column layout is head-major (m, d) -> m*32+d
    md = (np.arange(HEADS)[:, None] + 24 * np.arange(D)[None, :]).reshape(-1)
    q_rows, k_rows, v_rows = md + 0, md + 8, md + 16

    wq_full = np.ascontiguousarray((Wqkv[q_rows, :] * scale).T)   # [256, 256]
    bq_full = (bqkv[q_rows] * scale).reshape(-1, 1)
    wk_full = np.ascontiguousarray(Wqkv[k_rows, :].T)
    bk_full = bqkv[k_rows].reshape(-1, 1)
    w0_full = np.ascontiguousarray(W0.T)                          # [c-in, 256]

    per_mg = []
    for mg in range(2):
        hsl = slice(mg * 128, (mg + 1) * 128)
        # zero-interleaved layouts: per head 32 real rows/cols then 32
        # zeros, so K=64 / M=64 matmuls engage the full PE array
        wq = np.zeros((C, C), f)
        wk = np.zeros((C, C), f)
        bq = np.zeros((C, 1), f)
        bk = np.zeros((C, 1), f)
        wv = np.zeros((C + 1, NV), f)
        for m in range(MG):
            gm = mg * MG + m
            wq[:, m * 64:m * 64 + 32] = wq_full[:, gm * 32:(gm + 1) * 32]
            wk[:, m * 64:m * 64 + 32] = wk_full[:, gm * 32:(gm + 1) * 32]
            bq[m * 64:m * 64 + 32] = bq_full[gm * 32:(gm + 1) * 32]
            bk[m * 64:m * 64 + 32] = bk_full[gm * 32:(gm + 1) * 32]
            vr = v_rows[gm * D:(gm + 1) * D]
            wv[0:C, m * 64:m * 64 + 32] = Wqkv[vr, :].T
            wv[C, m * 64:m * 64 + 32] = bqkv[vr]
            wv[C, m * 64 + 32] = 1.0
        w0b = b0[None, :] if mg == 0 else np.zeros((1, C), f)
        per_mg.append({
            "wq": wq.astype(NPBF),
            "bq": bq,
            "wk": wk.astype(NPBF),
            "bk": bk,
            "wv": wv.astype(NPBF),
            "w0": np.ascontiguousarray(w0_full[hsl, :]).astype(NPBF),
            "w0b": np.ascontiguousarray(w0b).astype(NPBF),
        })

    maps = []
    for b in range(B):
        x_aug = np.concatenate([x[b].reshape(C, N), np.ones((1, N), f)], axis=0)
        x_bf = np.ascontiguousarray(x_aug).astype(NPBF)
        for mg in range(2):
            maps.append({"x": x_bf, **per_mg[mg]})
    return maps


def assemble_output(ys):
    out = np.empty((B, C, N), np.float32)
    for b in range(B):
        np.add(ys[2 * b], ys[2 * b + 1], out=out[b])
    return out.reshape(B, C, HH, WW)


def run(inputs, trace=False):
    nc = _get_program()
    maps = make_in_maps(**inputs)
    res = bass_utils.run_bass_kernel_spmd(
        nc, maps, core_ids=list(range(NCORES)), trace=trace
    )
    ys = [res.results[c]["y"] for c in range(NCORES)]
    return assemble_output(ys), res.exec_time_ns


def kernel(**inputs):
    out, _ = run(inputs, trace=False)
    return out


# revision 20
# speedup vs baseline: 1.3459x; 1.0554x over previous
"""Multi-head self-attention (1x1-conv QKV -> softmax attention -> 1x1-conv)
on Trainium2, 8 NeuronCores, sharded by (batch, head-group).

Problem (hardcoded): x[4,256,48,48], Wqkv[768,256], bqkv[768], W0[256,256],
b0[256]; heads=8, dim_head=32, n=2304 pixels.

Sharding: core = b*2 + mg, mg in {0,1} a group of 4 heads. Each core runs
QKV projection (its 4 heads), attention over all 2304 queries x 2304 keys,
and a PARTIAL output projection y_mg = W0[:, mg-heads] @ out_mg (+ b0 on
mg=0 only). The host sums the two partials per image — a pure elementwise
add during output assembly. No cross-core communication on device.

Per-core dataflow — all large matmuls in BF16 (1 cyc/row on the PE vs 4
for fp32; tolerance is 2e-2 so bf16 is plenty):
  - x_aug [257, 2304] bf16: image + ones row, DMA'd in 512-col chunks so
    projections start early.
  - k,q [(4 heads x 32)=128, 2304] bf16 (Wq, bq pre-scaled by d^-0.5 on
    host), vT [j, 4*(32+1)=132] bf16: per head 32 v-dims + ones col
    (bias + softmax denominator via the x ones-row / vt ones-col tricks).
  - scores^T S_T[j, i] per head pair: K=32 bf16 matmuls row-packed via
    tile_position; each matmul output owns a full PSUM bank. Queries in
    chunks (512,512,512,512,256) — wide chunks keep PE array utilization
    high (narrow ones let the HAM clock-gate re-throttle to 1.2 GHz).
  - P = exp(S_T), split across TWO engines per key-tile j:
      * ACT: table exp (exact), bf16 out
      * DVE: Schraudolph bit-trick exp targeting bf16 bits: one
        scalar_tensor_tensor (st*A16 + B16) -> int16 tile, bitcast bf16
        (max elementwise err ~3.4%; mostly cancels post-softmax).
  - out^T+den: the head PAIR's PV matmuls share ONE PSUM bank: a K=1
    "opener" matmul (start=True) writes a pattern row — 0.0 on the output
    rows 0:33/64:97, 1.0 on the junk rows — so both heads can accumulate
    with start=False (pending-zero bytes make their first write an
    overwrite) and the junk rows stay reciprocal-safe. All MMs of a group
    are chained with chain_iter_dep so the scheduler cannot move an
    accumulate past the closing stop.
  - HAM fillers: the PE clock-gate re-throttles unless the array stays
    busy; full-array K=128 filler matmuls over resident tiles run in the
    queue slots where the PV matmul would otherwise sit waiting, and each
    chunk's normalize is deferred into the next chunk's j-loop so the PE
    never drains at a chunk boundary.
  - normalize: dens sit at psum partitions 32/96. Two 32-row block copies
    to a base-0 tile (custom DVE ops need base 0), one
    reciprocal_approx_fast over 64 partitions, recips staged to bf16,
    then a K=64 bf16 mask matmul broadcasts recipA to partitions 0:32 and
    recipB to 64:96 of an rr PSUM bank; ACT stages rr to SBUF (DVE reads
    at most one PSUM operand) and one [128,w] DVE mul writes normalized
    bf16 outc.
  - y_mg = W0_mg @ outc (+ b0 via ones-row matmul on mg=0), fp32 out,
    DMA per chunk.
"""

import os as _os

import numpy as np
import ml_dtypes

import concourse.bass as bass
import concourse.mybir as mybir
import concourse.tile as tile
from concourse import bacc
from concourse import bass_utils

F32 = mybir.dt.float32
BF = mybir.dt.bfloat16
I16 = mybir.dt.int16
AF = mybir.ActivationFunctionType
ALU = mybir.AluOpType
NPBF = ml_dtypes.bfloat16

B, C, HH, WW = 4, 256, 48, 48
HEADS, D = 8, 32
N = HH * WW            # 2304 pixels = queries = keys per core
NCORES = 8
JT = N // 128          # 18 key tiles
MG = 4                 # heads per core
NV = MG * 64           # 256: vT cols/head: 32 v + ones + 31 zeros (PE-array padding)
ICW = 512              # query chunk tile width

# query chunks: wide and uniform; only one narrow tail (narrow chunks drop
# PE array utilization enough for the HAM clock-gate to re-throttle)
QCHUNKS = [(0, 480), (480, 480), (960, 480), (1440, 480), (1920, 384)]

# Schraudolph exp in bf16 bit space: exp(s) ~ bitcast_bf16(int16(A16*s + B16))
A16_SCHR = float((1 << 7) / np.log(2.0))         # 184.665
B16_SCHR = float((127 << 7) - 5.375)             # C16=5.375: ~3.4% max elem err

N_SPIN0 = int(_os.environ.get("KSPIN0", "16"))   # warmup matmuls before projections
N_FILLN = int(_os.environ.get("KFILL", "2"))     # filler matmuls before each PV pair
N_FILL = int(_os.environ.get("KFILLW", "512"))   # filler matmul free-dim cols

N_DVE = int(_os.environ.get("KDVE", "14"))       # of 2*JT=36 exp instrs on DVE
# keep early slots on ACT: each chunk's DVE queue first drains the previous
# chunk's normalize ops
DVE_SLOTS = frozenset(6 + ((2 * i + 1) * (2 * JT - 6)) // (2 * N_DVE) for i in range(N_DVE))
PV_LAG = 4                                       # pv emission lag (j-iterations)


def _chunks(total, step):
    out = []
    o = 0
    while o < total:
        w = min(step, total - o)
        out.append((o, w))
        o += w
    return out


def _body(tc, x_d, wq_d, bq_d, wk_d, bk_d, wv_d, w0_d, w0b_d, y_d):
    from contextlib import ExitStack

    nc = tc.nc
    with ExitStack() as ctx:
        const = ctx.enter_context(tc.tile_pool(name="const", bufs=1))
        data = ctx.enter_context(tc.tile_pool(name="data", bufs=1))

        # ---------------- load inputs ----------------
        x_sb = [const.tile([128, N], BF, name=f"xa{t}", tag=f"xa{t}") for t in range(2)]
        x1_sb = const.tile([1, N], BF, name="xones", tag="xones")
        for (o, w) in _chunks(N, N // 2):
            nc.sync.dma_start(x_sb[0][:, o:o + w], x_d[0:128, o:o + w])
            nc.sync.dma_start(x_sb[1][:, o:o + w], x_d[128:256, o:o + w])
        nc.gpsimd.dma_start(x1_sb[:], x_d[256:257, :])

        def load2(name, dram, cols, dt=BF):
            ts_ = [const.tile([128, cols], dt, name=f"{name}{t}", tag=f"{name}{t}") for t in range(2)]
            nc.sync.dma_start(ts_[0][:], dram[0:128, :])
            nc.sync.dma_start(ts_[1][:], dram[128:256, :])
            return ts_

        # wq/wk: [256 chan-in, 256 out] — per head 32 real cols + 32 zero
        # cols, so k/q carry zero-interleaved rows and the K=64 score
        # matmuls light the whole PE array (zeros add nothing)
        wq_sb = load2("wq", wq_d, C)
        wk_sb = load2("wk", wk_d, C)
        wv_sb = load2("wv", wv_d, NV)
        wv1_sb = const.tile([1, NV], BF, name="wvbias", tag="wvbias")
        nc.gpsimd.dma_start(wv1_sb[:], wv_d[256:257, :])
        w0_sb = const.tile([128, C], BF, name="w0", tag="w0")
        nc.sync.dma_start(w0_sb[:], w0_d[0:128, :])
        w01_sb = const.tile([1, C], BF, name="w0bias", tag="w0bias")
        nc.gpsimd.dma_start(w01_sb[:], w0b_d[0:1, :])
        bq_sb = load2("bq", bq_d, 1, dt=F32)
        bk_sb = load2("bk", bk_d, 1, dt=F32)

        ones_row = const.tile([1, N], BF, name="ones_row", tag="ones_row")
        nc.vector.memset(ones_row[:], 1.0)
        # Schraudolph additive constant, matching the exp input AP shape
        bexp = const.tile([128, 2 * ICW], F32, name="bexp", tag="bexp")
        nc.vector.memset(bexp[:], B16_SCHR)
        # bank-opener row: 0 over the PV output rows (0:33, 64:97), 1.0 over
        # the junk rows so the den blocks stay reciprocal-safe
        patt = const.tile([1, 128], BF, name="patt", tag="patt")
        nc.vector.memset(patt[:], 1.0)
        nc.vector.memset(patt[0:1, 0:33], 0.0)
        nc.vector.memset(patt[0:1, 64:97], 0.0)

        # persistent activations: pair tile g holds [kA, 0, kB, 0] rows
        k_sb = [data.tile([128, N], BF, name=f"k{g}", tag=f"k{g}") for g in range(2)]
        q_sb = [data.tile([128, N], BF, name=f"q{g}", tag=f"q{g}") for g in range(2)]
        vt_sb = [data.tile([128, NV], BF, name=f"vt{j}", tag=f"vt{j}") for j in range(JT)]
        # output tiles in pv layout: tile pr holds head 2*pr at partitions
        # 0-31 and head 2*pr+1 at partitions 64-95
        outc_sb = [data.tile([128, N], BF, name=f"oc{t}", tag=f"oc{t}") for t in range(2)]
        # out-projection staging: pv-layout outc remapped head-major via
        # SBUF->SBUF DMA, partial y = W0_mg @ od (+ b0 via ones-row on mg=0)
        od_sb = data.tile([128, N], BF, name="od", tag="od")
        y_sb = [data.tile([128, N], F32, name=f"y{g}", tag=f"y{g}") for g in range(2)]

        # ---------------- projections (bf16) ----------------
        with tc.tile_pool(name="prj", bufs=2, space="PSUM") as prj:
            # warm the PE during the x-DMA wait so projections start at
            # full clock: full-array matmuls into a scratch psum tile
            wt = prj.tile([128, 512], F32, name="wt", tag="wt")
            for i in range(N_SPIN0):
                o = 256 * (i & 1)
                nc.tensor.matmul(
                    wt[:, o:o + 256], wq_sb[0][:, 0:128], x_sb[0][:, 0:256],
                    start=True, stop=True, tile_position=(0, 0),
                )
            # interleave the dense 512-wide k/q chunks between the narrow
            # V-projection tiles so PE array utilization never dips long
            # enough for the HAM clock-gate to re-throttle
            def emit_kq(g, o, w):
                gsl = slice(g * 128, (g + 1) * 128)
                kps = prj.tile([128, 512], F32, name="kps", tag="kps")
                nc.tensor.matmul(kps[:, :w], wk_sb[0][:, gsl], x_sb[0][:, o:o + w], start=True, stop=False)
                nc.tensor.matmul(kps[:, :w], wk_sb[1][:, gsl], x_sb[1][:, o:o + w], start=False, stop=True)
                nc.scalar.activation(k_sb[g][:, o:o + w], kps[:, :w], AF.Identity, bias=bk_sb[g][:, 0:1])
                qps = prj.tile([128, 512], F32, name="qps", tag="qps")
                nc.tensor.matmul(qps[:, :w], wq_sb[0][:, gsl], x_sb[0][:, o:o + w], start=True, stop=False)
                nc.tensor.matmul(qps[:, :w], wq_sb[1][:, gsl], x_sb[1][:, o:o + w], start=False, stop=True)
                nc.scalar.activation(q_sb[g][:, o:o + w], qps[:, :w], AF.Identity, bias=bq_sb[g][:, 0:1])

            def emit_v(j):
                jsl = slice(j * 128, (j + 1) * 128)
                vps = prj.tile([128, NV], F32, name="vps", tag="vps")
                nc.tensor.matmul(vps[:], x_sb[0][:, jsl], wv_sb[0][:], start=True, stop=False)
                nc.tensor.matmul(vps[:], x_sb[1][:, jsl], wv_sb[1][:], start=False, stop=False)
                nc.tensor.matmul(vps[:], x1_sb[:, jsl], wv1_sb[:], start=False, stop=True)
                if j % 2 == 0:
                    nc.scalar.copy(vt_sb[j][:], vps[:])
                else:
                    nc.vector.tensor_copy(vt_sb[j][:], vps[:])

            kq_units = [(g, o, w) for g in range(2) for (o, w) in _chunks(N, 512)]
            vj = 0
            for i, (g, o, w) in enumerate(kq_units):
                emit_kq(g, o, w)
                while vj < JT and vj < 2 * (i + 1):
                    emit_v(vj)
                    vj += 1
            while vj < JT:
                emit_v(vj)
                vj += 1

        # ---------------- attention main loop ----------------
        # Both head pairs interleave in one j-loop: the other pair's
        # matmuls fill the PE queue slots where a lone pair would sit
        # waiting on exp, so the PE stays busy with real work and the HAM
        # clock-gate keeps the array at 2.4 GHz. Both exp engines (ACT
        # table exp / DVE Schraudolph) run every j.
        # PSUM: stp 3x2 banks (scores; rr matmuls borrow ring slots)
        # + pvp 2x1 (bank-shared pair each) = 8.
        with tc.tile_pool(name="stp", bufs=3, space="PSUM") as stp, \
             tc.tile_pool(name="pvp", bufs=1, space="PSUM") as pvp, \
             tc.tile_pool(name="ptp", bufs=12) as ptp, \
             tc.tile_pool(name="epi", bufs=4) as epi:

            pending = []   # deferred normalize muls: list of closures
            pending_dma = []   # od-remap DMAs for normalized chunks
            pending_fin = []   # out-projection matmuls for remapped chunks

            def flush_norm():
                while pending:
                    pending.pop(0)()

            def emit_od_dma(ic0, w):
                for p in range(2):
                    src_t = outc_sb[p]
                    nc.sync.dma_start(od_sb[p * 64:p * 64 + 32, ic0:ic0 + w], src_t[0:32, ic0:ic0 + w])
                    nc.sync.dma_start(od_sb[p * 64 + 32:p * 64 + 64, ic0:ic0 + w], src_t[64:96, ic0:ic0 + w])

            def emit_fin(ic0, w):
                for mt in range(2):
                    msl = slice(mt * 128, (mt + 1) * 128)
                    fps = stp.tile([128, 1024], F32, name="fps", tag="st")
                    nc.tensor.matmul(fps[:, :w], w0_sb[:, msl], od_sb[:, ic0:ic0 + w], start=True, stop=False)
                    nc.tensor.matmul(fps[:, :w], w01_sb[:, msl], ones_row[:, ic0:ic0 + w], start=False, stop=True)
                    nc.scalar.copy(y_sb[mt][:, ic0:ic0 + w], fps[:, :w])
                    nc.sync.dma_start(y_d[msl, ic0:ic0 + w], y_sb[mt][:, ic0:ic0 + w])

            for (ic0, w) in QCHUNKS:
                pvs = [pvp.tile([128, ICW], F32, name=f"pv{p}", tag=f"pv{p}") for p in range(2)]
                pts = {}

                def emit_pv(j, w=w, pvs=pvs, pts=pts, ic0=ic0):
                    for p in range(2):
                        pt = pts.pop((j, p))
                        for hl, base in ((0, 0), (1, 64)):
                            gh = 2 * p + hl
                            mi = nc.tensor.matmul(
                                pvs[p][base:base + 64, 0:w],
                                vt_sb[j][:, gh * 64:gh * 64 + 64],
                                pt[:].bitcast(BF)[:, hl * ICW:hl * ICW + w],
                                start=False,
                                stop=(j == JT - 1 and hl == 1),
                                tile_position=(0, base),
                            )
                            tc.chain_iter_dep(f"pvc{p}_{ic0}", mi.ins)

                for j in range(JT):
                    for p in range(2):
                        st = stp.tile([128, 1024], F32, name="st", tag="st")
                        for hl in range(2):
                            nc.tensor.matmul(
                                st[:, hl * 512:hl * 512 + w],
                                k_sb[p][hl * 64:(hl + 1) * 64, j * 128:(j + 1) * 128],
                                q_sb[p][hl * 64:(hl + 1) * 64, ic0:ic0 + w],
                                start=True, stop=True,
                                tile_position=(hl * 64, 0),
                            )
                        st_v = st[:].rearrange("p (s q) -> p s q", s=2)[:, :, 0:w]
                        if (2 * j + p) in DVE_SLOTS:
                            pt = ptp.tile([128, 2 * ICW], I16, name="pt", tag="pt")
                            nc.vector.scalar_tensor_tensor(
                                pt[:].rearrange("p (s q) -> p s q", s=2)[:, :, 0:w],
                                st_v, A16_SCHR,
                                bexp[:].rearrange("p (s q) -> p s q", s=2)[:, :, 0:w],
                                ALU.mult, ALU.add,
                            )
                        else:
                            pt = ptp.tile([128, 2 * ICW], BF, name="pt", tag="pt")
                            nc.scalar.activation(
                                pt[:].rearrange("p (s q) -> p s q", s=2)[:, :, 0:w],
                                st_v, AF.Exp,
                            )
                        pts[(j, p)] = pt
                    if j == 1:
                        # previous chunk's deferred normalize muls
                        flush_norm()
                    if j == 4 and pending_dma:
                        emit_od_dma(*pending_dma.pop(0))
                    if j == 12 and pending_fin:
                        emit_fin(*pending_fin.pop(0))
                    if j == 3:
                        # open the shared banks: K=1 matmuls write the
                        # pattern row to all 128 partitions, start=True.
                        # Late enough that the pool-release wait (previous
                        # chunk's normalize muls) is already satisfied.
                        for p in range(2):
                            mi = nc.tensor.matmul(
                                pvs[p][:, 0:w], patt[0:1, 0:128], ones_row[0:1, 0:w],
                                start=True, stop=False, tile_position=(0, 0),
                            )
                            tc.chain_iter_dep(f"pvc{p}_{ic0}", mi.ins)
                    if j >= PV_LAG:
                        emit_pv(j - PV_LAG)
                for j in range(JT - PV_LAG, JT):
                    emit_pv(j)

                # normalize (all DVE/no PSUM pressure): dens at psum
                # partitions 32/96, junk rows hold 1.0 from the opener (PV
                # zero-padding accumulates zeros onto them). Block copies
                # to a base-0 tile (custom DVE ops need base 0), one
                # reciprocal over 64 partitions, then intra-block
                # stream_shuffle broadcasts + two muls per pair. The muls
                # (which free the pv banks) are deferred into the next
                # chunk so the j-loop tail keeps both engines busy.
                for p in range(2):
                    pv = pvs[p]
                    oc = outc_sb[p]
                    dd = epi.tile([64, ICW], F32, name="dd", tag="dd")
                    rc = epi.tile([64, ICW], F32, name="rc", tag="rc")
                    rba = epi.tile([128, ICW], F32, name="rba", tag="rba")
                    nc.vector.tensor_copy(dd[0:32, 0:w], pv[32:64, 0:w])
                    nc.vector.tensor_copy(dd[32:64, 0:w], pv[96:128, 0:w])
                    nc.vector.reciprocal_approx_fast(rc[0:64, 0:w], dd[0:64, 0:w])
                    # one shuffle broadcasts within each 32-block: block 0
                    # -> recipA, block 1 -> recipB
                    nc.vector.stream_shuffle(rba[0:64, 0:w], rc[0:64, 0:w], [0] * 32)
                    nc.vector.tensor_copy(rba[64:96, 0:w], rba[32:64, 0:w])

                    def mul_norm(pv=pv, oc=oc, rba=rba, ic0=ic0, w=w):
                        # rows 33:64 multiply junk by recipB -> garbage rows
                        # the out-projection remap never reads
                        nc.vector.tensor_mul(oc[0:96, ic0:ic0 + w], pv[0:96, 0:w], rba[0:96, 0:w])
                    pending.append(mul_norm)
                pending_dma.append((ic0, w))
                pending_fin.append((ic0, w))
            flush_norm()
            while pending_dma:
                emit_od_dma(*pending_dma.pop(0))
            while pending_fin:
                emit_fin(*pending_fin.pop(0))



def build_program():
    nc = bacc.Bacc(
        "TRN2",
        target_bir_lowering=False,
        debug=False,
        enable_asserts=False,
        num_devices=NCORES,
    )
    x_d = nc.dram_tensor("x", [C + 1, N], BF, kind="ExternalInput").ap()
    wq_d = nc.dram_tensor("wq", [C, C], BF, kind="ExternalInput").ap()
    bq_d = nc.dram_tensor("bq", [C, 1], F32, kind="ExternalInput").ap()
    wk_d = nc.dram_tensor("wk", [C, C], BF, kind="ExternalInput").ap()
    bk_d = nc.dram_tensor("bk", [C, 1], F32, kind="ExternalInput").ap()
    wv_d = nc.dram_tensor("wv", [C + 1, NV], BF, kind="ExternalInput").ap()
    w0_d = nc.dram_tensor("w0", [128, C], BF, kind="ExternalInput").ap()
    w0b_d = nc.dram_tensor("w0b", [1, C], BF, kind="ExternalInput").ap()
    y_d = nc.dram_tensor("y", [C, N], F32, kind="ExternalOutput").ap()

    with tile.TileContext(nc) as tc:
        _body(tc, x_d, wq_d, bq_d, wk_d, bk_d, wv_d, w0_d, w0b_d, y_d)
    nc.compile()
    return nc


_CACHE = {}


def _get_program():
    if "nc" not in _CACHE:
        _CACHE["nc"] = build_program()
    return _CACHE["nc"]


def make_in_maps(x, Wqkv, bqkv, W0, b0):
    f = np.float32
    x = np.asarray(x, f)
    Wqkv = np.asarray(Wqkv, f)
    bqkv = np.asarray(bqkv, f)
    W0 = np.asarray(W0, f)
    b0 = np.asarray(b0, f)

    scale = f(D) ** f(-0.5)
    # channel o = d*24 + k*8 + m ; column layout is head-major (m, d) -> m*32+d
    md = (np.arange(HEADS)[:, None] + 24 * np.arange(D)[None, :]).reshape(-1)
    q_rows, k_rows, v_rows = md + 0, md + 8, md + 16

    wq_full = np.ascontiguousarray((Wqkv[q_rows, :] * scale).T)   # [256, 256]
    bq_full = (bqkv[q_rows] * scale).reshape(-1, 1)
    wk_full = np.ascontiguousarray(Wqkv[k_rows, :].T)
    bk_full = bqkv[k_rows].reshape(-1, 1)
    w0_full = np.ascontiguousarray(W0.T)                          # [c-in, 256]

    per_mg = []
    for mg in range(2):
        hsl = slice(mg * 128, (mg + 1) * 128)
        # zero-interleaved layouts: per head 32 real rows/cols then 32
        # zeros, so K=64 / M=64 matmuls engage the full PE array
        wq = np.zeros((C, C), f)
        wk = np.zeros((C, C), f)
        bq = np.zeros((C, 1), f)
        bk = np.zeros((C, 1), f)
        wv = np.zeros((C + 1, NV), f)
        for m in range(MG):
            gm = mg * MG + m
            wq[:, m * 64:m * 64 + 32] = wq_full[:, gm * 32:(gm + 1) * 32]
            wk[:, m * 64:m * 64 + 32] = wk_full[:, gm * 32:(gm + 1) * 32]
            bq[m * 64:m * 64 + 32] = bq_full[gm * 32:(gm + 1) * 32]
            bk[m * 64:m * 64 + 32] = bk_full[gm * 32:(gm + 1) * 32]
            vr = v_rows[gm * D:(gm + 1) * D]
            wv[0:C, m * 64:m * 64 + 32] = Wqkv[vr, :].T
            wv[C, m * 64:m * 64 + 32] = bqkv[vr]
            wv[C, m * 64 + 32] = 1.0
        w0b = b0[None, :] if mg == 0 else np.zeros((1, C), f)
        per_mg.append({
            "wq": wq.astype(NPBF),
            "bq": bq,
            "wk": wk.astype(NPBF),
            "bk": bk,
            "wv": wv.astype(NPBF),
            "w0": np.ascontiguousarray(w0_full[hsl, :]).astype(NPBF),
            "w0b": np.ascontiguousarray(w0b).astype(NPBF),
        })

    maps = []
    for b in range(B):
        x_aug = np.concatenate([x[b].reshape(C, N), np.ones((1, N), f)], axis=0)
        x_bf = np.ascontiguousarray(x_aug).astype(NPBF)
        for mg in range(2):
            maps.append({"x": x_bf, **per_mg[mg]})
    return maps


def assemble_output(ys):
    out = np.empty((B, C, N), np.float32)
    for b in range(B):
        np.add(ys[2 * b], ys[2 * b + 1], out=out[b])
    return out.reshape(B, C, HH, WW)


def run(inputs, trace=False):
    nc = _get_program()
    maps = make_in_maps(**inputs)
    res = bass_utils.run_bass_kernel_spmd(
        nc, maps, core_ids=list(range(NCORES)), trace=trace
    )
    ys = [res.results[c]["y"] for c in range(NCORES)]
    return assemble_output(ys), res.exec_time_ns


def kernel(**inputs):
    out, _ = run(inputs, trace=False)
    return out
